# revision 12
# baseline (speedup 1.0000x reference)
"""Transformer block (LN -> MHA -> residual -> LN -> MLP -> residual) on 8 trn2 cores.

Sharding: token-parallel. Core c owns 1024 tokens of batch b=c//2 (flattened
B*N = 8192 tokens / 8 cores). Attention needs full-sequence K/V, so each core
computes K/V for its batch's full 2048-token sequence (K/V projection is
duplicated across the 2 cores sharing a batch; ~12% extra FLOPs, zero
collectives). Key order within each core's context is permuted so its own
tokens come first -- softmax attention is permutation-invariant over keys, so
this is exact and lets all 8 cores run one identical program (SPMD, per-core
input maps).

Numerics: attention path matmuls in bf16 (inputs rounded, fp32 PSUM
accumulate), MLP/proj-adjacent big GEMMs in float32r (TF32-like, full fp32
storage, full PE rate). LayerNorm, softmax accumulation, residuals in fp32.
LN affine params are folded into the following weight matrices on the host
(exact); proj bias is folded into the residual input on the host.

Layout: activations feature-major ("transposed", [d, tokens]) for matmul
operands via PE-transpose after each LayerNorm; matmul outputs that feed
softmax/LN/residuals come out token-major. Softmax denominators ride along
the PV matmul as a ones-column appended to V (M=65); normalization uses a
K=1 ones-matmul to broadcast 1/denom across partitions.
"""

import numpy as np
import ml_dtypes

import concourse.bacc as bacc
import concourse.tile as tile
from concourse import mybir
from concourse.bass_utils import run_bass_kernel_spmd
from concourse.masks import make_identity

F32 = mybir.dt.float32
F32R = mybir.dt.float32r
BF16 = mybir.dt.bfloat16
AF = mybir.ActivationFunctionType
OP = mybir.AluOpType

D = 1024
H = 16
HD = 64
B = 4
N = 2048
DH = 4 * D
NCORES = 8
T_CTX = 2048
T_OWN = 1024
NT_CTX = T_CTX // 128  # 16
NT_OWN = T_OWN // 128  # 8
ND = D // 128  # 8
NH1 = DH // 128  # 32
EPS = 1e-5

_BUILD_CACHE = {}
_LAST_RESULTS = None  # BassKernelResults of the most recent run (for profiling)


def _layernorm_tiles(nc, work, const_eps, src_ap, out_tile):
    """LN stats + apply for one [128, D] token-major tile."""
    stats = work.tile([128, 2, 6], F32, tag="ln_stats", name="ln_stats")
    xg = src_ap.rearrange("p (g d) -> p g d", g=2)
    for g in range(2):
        nc.vector.bn_stats(out=stats[:, g, :], in_=xg[:, g, :])
    mv = work.tile([128, 2], F32, tag="ln_mv", name="ln_mv")
    nc.vector.bn_aggr(out=mv, in_=stats)
    std = work.tile([128, 1], F32, tag="ln_std", name="ln_std")
    nc.scalar.activation(out=std, in_=mv[:, 1:2], func=AF.Sqrt, bias=const_eps, scale=1.0)
    rstd = work.tile([128, 1], F32, tag="ln_rstd", name="ln_rstd")
    nc.vector.reciprocal(out=rstd, in_=std)
    nc.vector.tensor_scalar(
        out=out_tile,
        in0=src_ap,
        scalar1=mv[:, 0:1],
        scalar2=rstd,
        op0=OP.subtract,
        op1=OP.mult,
    )


def _build(has_qkv_bias: bool, phases: str = "0ABCD"):
    nc = bacc.Bacc("TRN2", target_bir_lowering=False, debug=False)

    x_ctx = nc.dram_tensor("x_ctx", [T_CTX, D], F32, kind="ExternalInput")
    xr = nc.dram_tensor("xr", [T_OWN, D], F32, kind="ExternalInput")
    wq_d = nc.dram_tensor("wq", [D, D], BF16, kind="ExternalInput")
    wk_d = nc.dram_tensor("wk", [D, D], BF16, kind="ExternalInput")
    wv_d = nc.dram_tensor("wv", [D, D], BF16, kind="ExternalInput")
    wp_d = nc.dram_tensor("wproj", [D, D], BF16, kind="ExternalInput")
    w1_d = nc.dram_tensor("w1", [D, DH], F32R, kind="ExternalInput")
    w2_d = nc.dram_tensor("w2", [DH, D], F32R, kind="ExternalInput")
    b1t_d = nc.dram_tensor("b1t", [128, NH1], F32, kind="ExternalInput")
    b2_d = nc.dram_tensor("b2", [1, D], F32R, kind="ExternalInput")
    if has_qkv_bias:
        bq_d = nc.dram_tensor("bq", [1, D], BF16, kind="ExternalInput")
        bk_d = nc.dram_tensor("bk", [1, D], BF16, kind="ExternalInput")
        bv_d = nc.dram_tensor("bv", [1, D], BF16, kind="ExternalInput")
    y_d = nc.dram_tensor("y", [T_OWN, D], F32, kind="ExternalOutput")

    wq_t = wq_d.ap().rearrange("(t p) o -> p t o", p=128)
    wk_t = wk_d.ap().rearrange("(t p) o -> p t o", p=128)
    wv_t = wv_d.ap().rearrange("(t p) o -> p t o", p=128)
    wp_t = wp_d.ap().rearrange("(t p) o -> p t o", p=128)
    w1_ap = w1_d.ap().rearrange("(t p) o -> p t o", p=128)
    w2_ap = w2_d.ap().rearrange("(t p) o -> p t o", p=128)

    with tile.TileContext(nc) as tc:
        const_cm = tc.tile_pool(name="const", bufs=1)
        const = const_cm.__enter__()
        eps_t = const.tile([128, 1], F32)
        nc.vector.memset(eps_t, EPS)
        ident = const.tile([128, 128], F32)
        make_identity(nc, ident)
        ones_f = const.tile([1, 128], F32)  # memset can't write f32r directly
        nc.vector.memset(ones_f, 1.0)
        ones_h = const.tile([1, HD], F32R)  # lhsT for 1/denom broadcast
        nc.vector.tensor_copy(out=ones_h, in_=ones_f[:, :HD])
        ones_tok = const.tile([1, 128], F32R)  # lhsT for fc2 bias init
        nc.vector.tensor_copy(out=ones_tok, in_=ones_f)
        b1t_sb = const.tile([128, NH1], F32)
        nc.sync.dma_start(out=b1t_sb, in_=b1t_d.ap())
        b2_sb = const.tile([1, D], F32R)
        nc.sync.dma_start(out=b2_sb, in_=b2_d.ap())
        if has_qkv_bias:
            ones_512b = const.tile([1, 512], BF16)
            nc.vector.memset(ones_512b, 1.0)
            ones_tokb = const.tile([1, 128], BF16)
            nc.vector.memset(ones_tokb, 1.0)
            bq_sb = const.tile([1, D], BF16)
            nc.sync.dma_start(out=bq_sb, in_=bq_d.ap())
            bk_sb = const.tile([1, D], BF16)
            nc.sync.dma_start(out=bk_sb, in_=bk_d.ap())
            bv_sb = const.tile([1, D], BF16)
            nc.sync.dma_start(out=bv_sb, in_=bv_d.ap())

        gpersist_cm = tc.tile_pool(name="gpersist", bufs=1)
        gpersist = gpersist_cm.__enter__()
        x2 = gpersist.tile([128, NT_OWN, D], F32)  # 32KB/part, phases B..E

        # xn1T + attnT live phases 0..B (released together, LIFO-nested)
        p_big1_cm = tc.tile_pool(name="p_big1", bufs=1)
        p_big1 = p_big1_cm.__enter__()
        xn1T = p_big1.tile([128, ND, T_CTX], BF16)  # 32KB/part
        attnT = p_big1.tile([128, ND, T_OWN], BF16)  # 16KB/part

        # ---------------- Phase 0: LN1 + transpose -> xn1T ----------------
        if "0" in phases:
         with (
            tc.tile_pool(name="w0", bufs=3) as w0,
            tc.tile_pool(name="ps0", bufs=4, space="PSUM") as ps0,
        ):
            for t in range(NT_CTX):
                xt = w0.tile([128, D], F32, tag="p0_x", name="p0_x")
                nc.sync.dma_start(out=xt, in_=x_ctx.ap()[t * 128 : (t + 1) * 128, :])
                xn = w0.tile([128, D], F32, tag="p0_xn", name="p0_xn")
                _layernorm_tiles(nc, w0, eps_t, xt, xn)
                for dt in range(ND):
                    tp = ps0.tile([128, 128], F32, tag="p0_tp", name="p0_tp")
                    nc.tensor.transpose(tp, xn[:, dt * 128 : (dt + 1) * 128], ident)
                    nc.vector.tensor_copy(
                        out=xn1T[:, dt, t * 128 : (t + 1) * 128], in_=tp
                    )

        # ---------------- Phase A: attention -> attnT ----------------
        if "A" in phases:
         with (
            tc.tile_pool(name="wA", bufs=1) as wA,
            tc.tile_pool(name="psA", bufs=1, space="PSUM") as psA,
        ):
            vt_tiles = [None] * NT_CTX
            for p in range(H // 2):
                g = p // 4
                if p % 4 == 0:
                    wv_g = wA.tile([128, ND, 512], BF16, tag="wv_g", name="wv_g")
                    nc.sync.dma_start(out=wv_g, in_=wv_t[:, :, g * 512 : (g + 1) * 512])
                    for kt_i in range(NT_CTX):
                        psv = psA.tile([128, 512], F32, tag="mm512", name="psv", bufs=3)
                        if has_qkv_bias:
                            nc.tensor.matmul(
                                psv,
                                ones_tokb,
                                bv_sb[:, g * 512 : (g + 1) * 512],
                                start=True,
                                stop=False,
                            )
                        for di in range(ND):
                            nc.tensor.matmul(
                                psv,
                                xn1T[:, di, kt_i * 128 : (kt_i + 1) * 128],
                                wv_g[:, di, :],
                                start=(di == 0 and not has_qkv_bias),
                                stop=(di == ND - 1),
                            )
                        vt = wA.tile(
                            [128, 8, 65], BF16, tag=f"vt{kt_i}", name=f"vt{kt_i}"
                        )
                        nc.vector.memset(vt[:, :, 64:65], 1.0)
                        nc.vector.tensor_copy(
                            out=vt[:, :, 0:64],
                            in_=psv.rearrange("p (h d) -> p h d", h=8),
                        )
                        vt_tiles[kt_i] = vt

                wk_p = wA.tile([128, ND, 128], BF16, tag="wk_p", name="wk_p", bufs=2)
                nc.sync.dma_start(out=wk_p, in_=wk_t[:, :, p * 128 : (p + 1) * 128])
                wq_p = wA.tile([128, ND, 128], BF16, tag="wq_p", name="wq_p", bufs=2)
                nc.sync.dma_start(out=wq_p, in_=wq_t[:, :, p * 128 : (p + 1) * 128])

                ktp = wA.tile([128, T_CTX], BF16, tag="ktp", name="ktp", bufs=2)
                for ch in range(T_CTX // 512):
                    psk = psA.tile([128, 512], F32, tag="mm512", name="psk", bufs=3)
                    if has_qkv_bias:
                        nc.tensor.matmul(
                            psk,
                            bk_sb[:, p * 128 : (p + 1) * 128],
                            ones_512b,
                            start=True,
                            stop=False,
                        )
                    for di in range(ND):
                        nc.tensor.matmul(
                            psk,
                            wk_p[:, di, :],
                            xn1T[:, di, ch * 512 : (ch + 1) * 512],
                            start=(di == 0 and not has_qkv_bias),
                            stop=(di == ND - 1),
                        )
                    nc.vector.tensor_copy(out=ktp[:, ch * 512 : (ch + 1) * 512], in_=psk)

                qtp = wA.tile([128, T_OWN], BF16, tag="qtp", name="qtp", bufs=2)
                for ch in range(T_OWN // 512):
                    psq = psA.tile([128, 512], F32, tag="mm512", name="psq", bufs=3)
                    if has_qkv_bias:
                        nc.tensor.matmul(
                            psq,
                            bq_sb[:, p * 128 : (p + 1) * 128],
                            ones_512b,
                            start=True,
                            stop=False,
                        )
                    for di in range(ND):
                        nc.tensor.matmul(
                            psq,
                            wq_p[:, di, :],
                            xn1T[:, di, ch * 512 : (ch + 1) * 512],
                            start=(di == 0 and not has_qkv_bias),
                            stop=(di == ND - 1),
                        )
                    nc.vector.tensor_copy(out=qtp[:, ch * 512 : (ch + 1) * 512], in_=psq)

                for qc in range(T_OWN // 512):
                    q0 = qc * 512
                    ov = [
                        psA.tile([65, 512], F32, tag=f"ov{h}", name=f"ov{h}", bufs=2)
                        for h in range(2)
                    ]
                    for kt_i in range(NT_CTX):
                        pts = []
                        for h in range(2):
                            st = psA.tile(
                                [128, 512], F32, tag="mm512", name="st", bufs=3
                            )
                            nc.tensor.matmul(
                                st,
                                ktp[h * 64 : (h + 1) * 64, kt_i * 128 : (kt_i + 1) * 128],
                                qtp[h * 64 : (h + 1) * 64, q0 : q0 + 512],
                                start=True,
                                stop=True,
                            )
                            pt = wA.tile(
                                [128, 512], BF16, tag=f"pt{h}", name=f"pt{h}", bufs=3
                            )
                            nc.scalar.activation(out=pt, in_=st, func=AF.Exp, scale=0.125)
                            pts.append(pt)
                        for h in range(2):
                            nc.tensor.matmul(
                                ov[h],
                                vt_tiles[kt_i][:, 2 * (p % 4) + h, :],
                                pts[h],
                                start=(kt_i == 0),
                                stop=(kt_i == NT_CTX - 1),
                            )
                    for h in range(2):
                        ovsb = wA.tile([65, 512], F32, tag="ovsb", name="ovsb", bufs=2)
                        nc.vector.tensor_copy(out=ovsb, in_=ov[h])
                        rec = wA.tile([1, 512], F32R, tag="rec", name="rec", bufs=2)
                        with nc.allow_low_precision(reason="f32r softmax denom"):
                            nc.vector.reciprocal(out=rec, in_=ovsb[64:65, :])
                        bc = psA.tile([64, 512], F32, tag="bc", name="bc", bufs=1)
                        nc.tensor.matmul(bc, ones_h, rec, start=True, stop=True)
                        nc.vector.tensor_tensor(
                            out=attnT[h * 64 : (h + 1) * 64, p, q0 : q0 + 512],
                            in0=ovsb[0:64, :],
                            in1=bc,
                            op=OP.mult,
                        )


        # ---------------- Phase B: proj + residual -> x2 ----------------
        if "B" in phases:
         with (
            tc.tile_pool(name="wB", bufs=3) as wB,
            tc.tile_pool(name="psB", bufs=4, space="PSUM") as psB,
        ):
            wp_sb = wB.tile([128, ND, D], BF16, tag="wp_sb", name="wp_sb", bufs=1)
            nc.sync.dma_start(out=wp_sb, in_=wp_t)
            for tt in range(NT_OWN):
                xr_t = wB.tile([128, D], F32, tag="xr_t", name="xr_t")
                nc.sync.dma_start(out=xr_t, in_=xr.ap()[tt * 128 : (tt + 1) * 128, :])
                for ch in range(2):
                    psb = psB.tile([128, 512], F32, tag="psb", name="psb")
                    for di in range(ND):
                        nc.tensor.matmul(
                            psb,
                            attnT[:, di, tt * 128 : (tt + 1) * 128],
                            wp_sb[:, di, ch * 512 : (ch + 1) * 512],
                            start=(di == 0),
                            stop=(di == ND - 1),
                        )
                    nc.vector.tensor_tensor(
                        out=x2[:, tt, ch * 512 : (ch + 1) * 512],
                        in0=psb,
                        in1=xr_t[:, ch * 512 : (ch + 1) * 512],
                        op=OP.add,
                    )

        p_big1_cm.__exit__(None, None, None)  # free xn1T + attnT

        # ---------------- Phase C: LN2 + transpose -> xn2T ----------------
        p_xn2_cm = tc.tile_pool(name="p_xn2", bufs=1)
        p_xn2 = p_xn2_cm.__enter__()
        xn2T = p_xn2.tile([128, ND, T_OWN], F32R)  # 32KB/part

        if "C" in phases:
         with (
            tc.tile_pool(name="wC", bufs=3) as wC,
            tc.tile_pool(name="psC", bufs=4, space="PSUM") as psC,
         ):
            for tt in range(NT_OWN):
                xn2 = wC.tile([128, D], F32, tag="p2_xn", name="p2_xn")
                _layernorm_tiles(nc, wC, eps_t, x2[:, tt, :], xn2)
                for dt in range(ND):
                    tp2 = psC.tile([128, 128], F32, tag="p2_tp", name="p2_tp")
                    nc.tensor.transpose(tp2, xn2[:, dt * 128 : (dt + 1) * 128], ident)
                    nc.vector.tensor_copy(
                        out=xn2T[:, dt, tt * 128 : (tt + 1) * 128], in_=tp2
                    )

        # ---------------- Phase D: MLP + residual -> y ----------------
        if "D" in phases:
         with (
            tc.tile_pool(name="wD", bufs=3) as wD,
            tc.tile_pool(name="h1pool", bufs=1) as h1pool,
            tc.tile_pool(name="psD1", bufs=3, space="PSUM") as psD1,
            tc.tile_pool(name="psD2", bufs=1, space="PSUM") as psD2,
        ):
            for c2 in range(2):
                t0 = c2 * 512
                h1 = h1pool.tile([128, NH1, 512], F32R, tag="h1", name="h1")
                for ht in range(NH1):
                    w1s = wD.tile([128, ND, 128], F32R, tag="w1s", name="w1s")
                    nc.sync.dma_start(
                        out=w1s, in_=w1_ap[:, :, ht * 128 : (ht + 1) * 128]
                    )
                    psh = psD1.tile([128, 512], F32, tag="psh", name="psh")
                    for di in range(ND):
                        nc.tensor.matmul(
                            psh,
                            w1s[:, di, :],
                            xn2T[:, di, t0 : t0 + 512],
                            start=(di == 0),
                            stop=(di == ND - 1),
                        )
                    nc.scalar.activation(
                        out=h1[:, ht, :],
                        in_=psh,
                        func=AF.Gelu,
                        bias=b1t_sb[:, ht : ht + 1],
                        scale=1.0,
                    )
                for dch in range(2):
                    out_ps = [
                        psD2.tile([128, 512], F32, tag=f"o{i}", name=f"o{i}")
                        for i in range(4)
                    ]
                    for ts in range(4):
                        nc.tensor.matmul(
                            out_ps[ts],
                            ones_tok,
                            b2_sb[:, dch * 512 : (dch + 1) * 512],
                            start=True,
                            stop=False,
                        )
                    for ht in range(NH1):
                        w2s = wD.tile([128, 512], F32R, tag="w2s", name="w2s")
                        nc.sync.dma_start(
                            out=w2s, in_=w2_ap[:, ht, dch * 512 : (dch + 1) * 512]
                        )
                        for ts in range(4):
                            nc.tensor.matmul(
                                out_ps[ts],
                                h1[:, ht, ts * 128 : (ts + 1) * 128],
                                w2s,
                                start=False,
                                stop=(ht == NH1 - 1),
                            )
                    for ts in range(4):
                        tt = c2 * 4 + ts
                        yt = wD.tile([128, 512], F32, tag="yt", name="yt")
                        nc.vector.tensor_tensor(
                            out=yt,
                            in0=out_ps[ts],
                            in1=x2[:, tt, dch * 512 : (dch + 1) * 512],
                            op=OP.add,
                        )
                        nc.sync.dma_start(
                            out=y_d.ap()[
                                tt * 128 : (tt + 1) * 128,
                                dch * 512 : (dch + 1) * 512,
                            ],
                            in_=yt,
                        )

        p_xn2_cm.__exit__(None, None, None)
        gpersist_cm.__exit__(None, None, None)
        const_cm.__exit__(None, None, None)

    nc.compile()
    return nc


def _get_nc(has_qkv_bias: bool):
    key = ("v2", has_qkv_bias)
    if key not in _BUILD_CACHE:
        _BUILD_CACHE[key] = _build(has_qkv_bias)
    return _BUILD_CACHE[key]


def kernel(x, w_qkv, w_proj, b_proj, w1, b1, w2, b2, g1, be1, g2, be2, **_):
    x = np.ascontiguousarray(np.asarray(x, dtype=np.float32))
    w_qkv = np.asarray(w_qkv, dtype=np.float32)
    w_proj = np.asarray(w_proj, dtype=np.float32)
    b_proj = np.asarray(b_proj, dtype=np.float32)
    w1 = np.asarray(w1, dtype=np.float32)
    b1 = np.asarray(b1, dtype=np.float32)
    w2 = np.ascontiguousarray(np.asarray(w2, dtype=np.float32))
    b2 = np.asarray(b2, dtype=np.float32)
    g1 = np.asarray(g1, dtype=np.float32)
    be1 = np.asarray(be1, dtype=np.float32)
    g2 = np.asarray(g2, dtype=np.float32)
    be2 = np.asarray(be2, dtype=np.float32)

    # fold LN affines into following matmuls (exact)
    wqkv_eff = w_qkv * g1[:, None]
    qkv_bias = be1 @ w_qkv
    bf = ml_dtypes.bfloat16
    wq = np.ascontiguousarray(wqkv_eff[:, :D].astype(bf))
    wk = np.ascontiguousarray(wqkv_eff[:, D : 2 * D].astype(bf))
    wv = np.ascontiguousarray(wqkv_eff[:, 2 * D :].astype(bf))
    wp_b = np.ascontiguousarray(w_proj.astype(bf))
    w1_eff = np.ascontiguousarray(w1 * g2[:, None])
    b1_eff = b1 + be2 @ w1
    b1t = np.ascontiguousarray(b1_eff.reshape(NH1, 128).T)
    b2r = np.ascontiguousarray(b2[None, :])
    has_qkv_bias = bool(np.any(qkv_bias != 0.0))

    nc = _get_nc(has_qkv_bias)

    shared = {
        "wq": wq,
        "wk": wk,
        "wv": wv,
        "wproj": wp_b,
        "w1": w1_eff,
        "w2": w2,
        "b1t": b1t,
        "b2": b2r,
    }
    if has_qkv_bias:
        shared["bq"] = np.ascontiguousarray(qkv_bias[None, :D].astype(bf))
        shared["bk"] = np.ascontiguousarray(qkv_bias[None, D : 2 * D].astype(bf))
        shared["bv"] = np.ascontiguousarray(qkv_bias[None, 2 * D :].astype(bf))

    in_maps = []
    for c in range(NCORES):
        b = c // 2
        half = c % 2
        own = x[b, half * T_OWN : (half + 1) * T_OWN, :]
        other = x[b, (1 - half) * T_OWN : (2 - half) * T_OWN, :]
        x_ctx = np.ascontiguousarray(np.concatenate([own, other], axis=0))
        xr_v = np.ascontiguousarray(own + b_proj[None, :])
        in_maps.append({"x_ctx": x_ctx, "xr": xr_v, **shared})

    res = run_bass_kernel_spmd(nc, in_maps, core_ids=list(range(NCORES)))
    global _LAST_RESULTS
    _LAST_RESULTS = res

    out = np.empty((B, N, D), dtype=np.float32)
    for c in range(NCORES):
        b = c // 2
        half = c % 2
        out[b, half * T_OWN : (half + 1) * T_OWN, :] = res.results[c]["y"]
    return out


# revision 18
# speedup vs baseline: 1.4732x; 1.4732x over previous
"""Transformer block (LN -> MHA -> residual -> LN -> MLP -> residual) on 8 trn2 cores.

Sharding: token-parallel. Core c owns 1024 tokens of batch b=c//2 (flattened
B*N = 8192 tokens / 8 cores). Attention needs full-sequence K/V, so each core
computes K/V for its batch's full 2048-token sequence (K/V projection is
duplicated across the 2 cores sharing a batch; ~12% extra FLOPs, zero
collectives). Key order within each core's context is permuted so its own
tokens come first -- softmax attention is permutation-invariant over keys, so
this is exact and lets all 8 cores run one identical program (SPMD, per-core
input maps).

Numerics: attention path matmuls in bf16 (inputs rounded, fp32 PSUM
accumulate), MLP/proj-adjacent big GEMMs in float32r (TF32-like, full fp32
storage, full PE rate). LayerNorm, softmax accumulation, residuals in fp32.
LN affine params are folded into the following weight matrices on the host
(exact); proj bias is folded into the residual input on the host.

Layout: activations feature-major ("transposed", [d, tokens]) for matmul
operands via PE-transpose after each LayerNorm; matmul outputs that feed
softmax/LN/residuals come out token-major. Softmax denominators ride along
the PV matmul as a ones-column appended to V (M=65); normalization uses a
K=1 ones-matmul to broadcast 1/denom across partitions.
"""

import numpy as np
import ml_dtypes

import concourse.bacc as bacc
import concourse.tile as tile
from concourse import mybir
from concourse.bass_utils import run_bass_kernel_spmd
from concourse.masks import make_identity

F32 = mybir.dt.float32
F32R = mybir.dt.float32r
BF16 = mybir.dt.bfloat16
AF = mybir.ActivationFunctionType
OP = mybir.AluOpType

D = 1024
H = 16
HD = 64
B = 4
N = 2048
DH = 4 * D
NCORES = 8
T_CTX = 2048
T_OWN = 1024
NT_CTX = T_CTX // 128  # 16
NT_OWN = T_OWN // 128  # 8
ND = D // 128  # 8
NH1 = DH // 128  # 32
EPS = 1e-5

_BUILD_CACHE = {}
_LAST_RESULTS = None  # BassKernelResults of the most recent run (for profiling)


def _layernorm_tiles(nc, work, const_eps, src_ap, out_tile):
    """LN stats + apply for one [128, D] token-major tile."""
    stats = work.tile([128, 2, 6], F32, tag="ln_stats", name="ln_stats")
    xg = src_ap.rearrange("p (g d) -> p g d", g=2)
    for g in range(2):
        nc.vector.bn_stats(out=stats[:, g, :], in_=xg[:, g, :])
    mv = work.tile([128, 2], F32, tag="ln_mv", name="ln_mv")
    nc.vector.bn_aggr(out=mv, in_=stats)
    std = work.tile([128, 1], F32, tag="ln_std", name="ln_std")
    nc.scalar.activation(out=std, in_=mv[:, 1:2], func=AF.Sqrt, bias=const_eps, scale=1.0)
    rstd = work.tile([128, 1], F32, tag="ln_rstd", name="ln_rstd")
    nc.vector.reciprocal(out=rstd, in_=std)
    nc.vector.tensor_scalar(
        out=out_tile,
        in0=src_ap,
        scalar1=mv[:, 0:1],
        scalar2=rstd,
        op0=OP.subtract,
        op1=OP.mult,
    )


def _build(has_qkv_bias: bool, phases: str = "0ABCD"):
    nc = bacc.Bacc("TRN2", target_bir_lowering=False, debug=False)

    x_ctx = nc.dram_tensor("x_ctx", [T_CTX, D], F32, kind="ExternalInput")
    wq_d = nc.dram_tensor("wq", [D, D], BF16, kind="ExternalInput")
    wk_d = nc.dram_tensor("wk", [D, D], BF16, kind="ExternalInput")
    wv_d = nc.dram_tensor("wv", [D, D], BF16, kind="ExternalInput")
    wp_d = nc.dram_tensor("wproj", [D, D], BF16, kind="ExternalInput")
    w1_d = nc.dram_tensor("w1", [D, DH], BF16, kind="ExternalInput")
    w2_d = nc.dram_tensor("w2", [DH, D], BF16, kind="ExternalInput")
    b1t_d = nc.dram_tensor("b1t", [128, NH1], F32, kind="ExternalInput")
    b2_d = nc.dram_tensor("b2", [1, D], F32R, kind="ExternalInput")
    bp_d = nc.dram_tensor("bp", [1, D], F32R, kind="ExternalInput")
    if has_qkv_bias:
        bq_d = nc.dram_tensor("bq", [1, D], BF16, kind="ExternalInput")
        bk_d = nc.dram_tensor("bk", [1, D], BF16, kind="ExternalInput")
        bv_d = nc.dram_tensor("bv", [1, D], BF16, kind="ExternalInput")
    y_d = nc.dram_tensor("y", [T_OWN, D], F32, kind="ExternalOutput")

    wq_t = wq_d.ap().rearrange("(t p) o -> p t o", p=128)
    wk_t = wk_d.ap().rearrange("(t p) o -> p t o", p=128)
    wv_t = wv_d.ap().rearrange("(t p) o -> p t o", p=128)
    wp_t = wp_d.ap().rearrange("(t p) o -> p t o", p=128)
    w1_ap = w1_d.ap().rearrange("(t p) o -> p t o", p=128)
    w2_ap = w2_d.ap().rearrange("(t p) o -> p t o", p=128)

    with tile.TileContext(nc) as tc:
        const_cm = tc.tile_pool(name="const", bufs=1)
        const = const_cm.__enter__()
        eps_t = const.tile([128, 1], F32)
        nc.vector.memset(eps_t, EPS)
        ident = const.tile([128, 128], F32)
        make_identity(nc, ident)
        ones_f = const.tile([1, 128], F32)  # memset can't write f32r directly
        nc.vector.memset(ones_f, 1.0)
        ones_h = const.tile([1, HD], F32R)  # lhsT for 1/denom broadcast
        nc.vector.tensor_copy(out=ones_h, in_=ones_f[:, :HD])
        ones_tok = const.tile([1, 128], F32R)  # lhsT for fc2 bias init
        nc.vector.tensor_copy(out=ones_tok, in_=ones_f)
        b1t_sb = const.tile([128, NH1], F32)
        nc.sync.dma_start(out=b1t_sb, in_=b1t_d.ap())
        b2_sb = const.tile([1, D], F32R)
        nc.sync.dma_start(out=b2_sb, in_=b2_d.ap())
        bp_sb = const.tile([1, D], F32R)
        nc.sync.dma_start(out=bp_sb, in_=bp_d.ap())
        if has_qkv_bias:
            ones_512b = const.tile([1, 512], BF16)
            nc.vector.memset(ones_512b, 1.0)
            ones_tokb = const.tile([1, 128], BF16)
            nc.vector.memset(ones_tokb, 1.0)
            bq_sb = const.tile([1, D], BF16)
            nc.sync.dma_start(out=bq_sb, in_=bq_d.ap())
            bk_sb = const.tile([1, D], BF16)
            nc.sync.dma_start(out=bk_sb, in_=bk_d.ap())
            bv_sb = const.tile([1, D], BF16)
            nc.sync.dma_start(out=bv_sb, in_=bv_d.ap())

        gpersist_cm = tc.tile_pool(name="gpersist", bufs=1)
        gpersist = gpersist_cm.__enter__()
        x2 = gpersist.tile([128, NT_OWN, D], F32)  # 32KB/part, phases B..E

        # xn1T + attnT live phases 0..B (released together, LIFO-nested)
        p_big1_cm = tc.tile_pool(name="p_big1", bufs=1)
        p_big1 = p_big1_cm.__enter__()
        xn1T = p_big1.tile([128, ND, T_CTX], BF16)  # 32KB/part
        attnT = p_big1.tile([128, ND, T_OWN], BF16)  # 16KB/part

        # ---------------- Phase 0: LN1 + transpose -> xn1T ----------------
        if "0" in phases:
         with (
            tc.tile_pool(name="w0", bufs=3) as w0,
            tc.tile_pool(name="ps0", bufs=4, space="PSUM") as ps0,
        ):
            for t in range(NT_CTX):
                xt = w0.tile([128, D], F32, tag="p0_x", name="p0_x")
                nc.sync.dma_start(out=xt, in_=x_ctx.ap()[t * 128 : (t + 1) * 128, :])
                xn = w0.tile([128, D], F32, tag="p0_xn", name="p0_xn")
                _layernorm_tiles(nc, w0, eps_t, xt, xn)
                for dt in range(ND):
                    tp = ps0.tile([128, 128], F32, tag="p0_tp", name="p0_tp")
                    nc.tensor.transpose(tp, xn[:, dt * 128 : (dt + 1) * 128], ident)
                    dst = xn1T[:, dt, t * 128 : (t + 1) * 128]
                    if dt % 2 == 0:
                        nc.vector.tensor_copy(out=dst, in_=tp)
                    else:
                        nc.scalar.copy(out=dst, in_=tp)

        # ---------------- Phase A: attention -> attnT ----------------
        if "A" in phases:
         with (
            tc.tile_pool(name="wA", bufs=1) as wA,
            tc.tile_pool(name="psA", bufs=1, space="PSUM") as psA,
        ):
            vt_tiles = [None] * NT_CTX
            for p in range(H // 2):
                g = p // 4
                if p % 4 == 0:
                    wv_g = wA.tile([128, ND, 512], BF16, tag="wv_g", name="wv_g")
                    nc.sync.dma_start(out=wv_g, in_=wv_t[:, :, g * 512 : (g + 1) * 512])
                    for kt_i in range(NT_CTX):
                        psv = psA.tile([128, 512], F32, tag="mm512", name="psv", bufs=2)
                        if has_qkv_bias:
                            nc.tensor.matmul(
                                psv,
                                ones_tokb,
                                bv_sb[:, g * 512 : (g + 1) * 512],
                                start=True,
                                stop=False,
                            )
                        for di in range(ND):
                            nc.tensor.matmul(
                                psv,
                                xn1T[:, di, kt_i * 128 : (kt_i + 1) * 128],
                                wv_g[:, di, :],
                                start=(di == 0 and not has_qkv_bias),
                                stop=(di == ND - 1),
                            )
                        vt = wA.tile(
                            [128, 8, 65], BF16, tag=f"vt{kt_i}", name=f"vt{kt_i}"
                        )
                        nc.vector.memset(vt[:, :, 64:65], 1.0)
                        nc.vector.tensor_copy(
                            out=vt[:, :, 0:64],
                            in_=psv.rearrange("p (h d) -> p h d", h=8),
                        )
                        vt_tiles[kt_i] = vt

                wk_p = wA.tile([128, ND, 128], BF16, tag="wk_p", name="wk_p", bufs=2)
                nc.sync.dma_start(out=wk_p, in_=wk_t[:, :, p * 128 : (p + 1) * 128])
                wq_p = wA.tile([128, ND, 128], BF16, tag="wq_p", name="wq_p", bufs=2)
                nc.sync.dma_start(out=wq_p, in_=wq_t[:, :, p * 128 : (p + 1) * 128])

                ktp = wA.tile([128, T_CTX], BF16, tag="ktp", name="ktp", bufs=2)
                for ch in range(T_CTX // 512):
                    psk = psA.tile([128, 512], F32, tag="mm512", name="psk", bufs=2)
                    if has_qkv_bias:
                        nc.tensor.matmul(
                            psk,
                            bk_sb[:, p * 128 : (p + 1) * 128],
                            ones_512b,
                            start=True,
                            stop=False,
                        )
                    for di in range(ND):
                        nc.tensor.matmul(
                            psk,
                            wk_p[:, di, :],
                            xn1T[:, di, ch * 512 : (ch + 1) * 512],
                            start=(di == 0 and not has_qkv_bias),
                            stop=(di == ND - 1),
                        )
                    nc.vector.tensor_copy(out=ktp[:, ch * 512 : (ch + 1) * 512], in_=psk)

                qtp = wA.tile([128, T_OWN], BF16, tag="qtp", name="qtp", bufs=2)
                for ch in range(T_OWN // 512):
                    psq = psA.tile([128, 512], F32, tag="mm512", name="psq", bufs=2)
                    if has_qkv_bias:
                        nc.tensor.matmul(
                            psq,
                            bq_sb[:, p * 128 : (p + 1) * 128],
                            ones_512b,
                            start=True,
                            stop=False,
                        )
                    for di in range(ND):
                        nc.tensor.matmul(
                            psq,
                            wq_p[:, di, :],
                            xn1T[:, di, ch * 512 : (ch + 1) * 512],
                            start=(di == 0 and not has_qkv_bias),
                            stop=(di == ND - 1),
                        )
                    nc.vector.tensor_copy(out=qtp[:, ch * 512 : (ch + 1) * 512], in_=psq)

                for qc in range(T_OWN // 512):
                    q0 = qc * 512
                    ov = [
                        psA.tile([65, 512], F32, tag=f"ov{h}", name=f"ov{h}", bufs=1)
                        for h in range(2)
                    ]

                    def _s_exp(kt_i):
                        st = psA.tile(
                            [128, 2, 512], F32, tag="stpair", name="st", bufs=2
                        )
                        for h in range(2):
                            nc.tensor.matmul(
                                st[:, h, :],
                                ktp[h * 64 : (h + 1) * 64, kt_i * 128 : (kt_i + 1) * 128],
                                qtp[h * 64 : (h + 1) * 64, q0 : q0 + 512],
                                start=True,
                                stop=True,
                            )
                        ptm = wA.tile(
                            [128, 2, 512], BF16, tag="ptm", name="ptm", bufs=4
                        )
                        nc.scalar.activation(out=ptm, in_=st, func=AF.Exp, scale=0.125)
                        return [ptm[:, 0, :], ptm[:, 1, :]]

                    # software-pipeline: keep one S/exp in flight ahead of PV
                    # so the PE never stalls on the ACT exp of the current tile
                    pts_prev = _s_exp(0)
                    for kt_i in range(NT_CTX):
                        pts_next = _s_exp(kt_i + 1) if kt_i + 1 < NT_CTX else None
                        for h in range(2):
                            nc.tensor.matmul(
                                ov[h],
                                vt_tiles[kt_i][:, 2 * (p % 4) + h, :],
                                pts_prev[h],
                                start=(kt_i == 0),
                                stop=(kt_i == NT_CTX - 1),
                            )
                        pts_prev = pts_next
                    for h in range(2):
                        ovsb = wA.tile([65, 512], F32, tag="ovsb", name="ovsb", bufs=2)
                        nc.vector.tensor_copy(out=ovsb, in_=ov[h])
                        rec = wA.tile([1, 512], F32R, tag="rec", name="rec", bufs=2)
                        with nc.allow_low_precision(reason="f32r softmax denom"):
                            nc.vector.reciprocal(out=rec, in_=ovsb[64:65, :])
                        bc = psA.tile([64, 512], F32, tag="mm512", name="bc", bufs=2)
                        nc.tensor.matmul(bc, ones_h, rec, start=True, stop=True)
                        nc.vector.tensor_tensor(
                            out=attnT[h * 64 : (h + 1) * 64, p, q0 : q0 + 512],
                            in0=ovsb[0:64, :],
                            in1=bc,
                            op=OP.mult,
                        )


        # ---------------- Phase B: proj + residual -> x2 ----------------
        if "B" in phases:
         with (
            tc.tile_pool(name="wB", bufs=3) as wB,
            tc.tile_pool(name="psB", bufs=4, space="PSUM") as psB,
        ):
            wp_sb = wB.tile([128, ND, D], BF16, tag="wp_sb", name="wp_sb", bufs=1)
            nc.sync.dma_start(out=wp_sb, in_=wp_t)
            for tt in range(NT_OWN):
                xr_t = wB.tile([128, D], F32, tag="xr_t", name="xr_t")
                nc.sync.dma_start(
                    out=xr_t, in_=x_ctx.ap()[tt * 128 : (tt + 1) * 128, :]
                )
                for ch in range(2):
                    psb = psB.tile([128, 512], F32, tag="psb", name="psb")
                    nc.tensor.matmul(
                        psb,
                        ones_tok,
                        bp_sb[:, ch * 512 : (ch + 1) * 512],
                        start=True,
                        stop=False,
                    )
                    for di in range(ND):
                        nc.tensor.matmul(
                            psb,
                            attnT[:, di, tt * 128 : (tt + 1) * 128],
                            wp_sb[:, di, ch * 512 : (ch + 1) * 512],
                            start=False,
                            stop=(di == ND - 1),
                        )
                    nc.vector.tensor_tensor(
                        out=x2[:, tt, ch * 512 : (ch + 1) * 512],
                        in0=psb,
                        in1=xr_t[:, ch * 512 : (ch + 1) * 512],
                        op=OP.add,
                    )

        p_big1_cm.__exit__(None, None, None)  # free xn1T + attnT

        # ---------------- Phase C: LN2 + transpose -> xn2T ----------------
        p_xn2_cm = tc.tile_pool(name="p_xn2", bufs=1)
        p_xn2 = p_xn2_cm.__enter__()
        xn2T = p_xn2.tile([128, ND, T_OWN], BF16)  # 16KB/part

        if "C" in phases:
         with (
            tc.tile_pool(name="wC", bufs=3) as wC,
            tc.tile_pool(name="psC", bufs=4, space="PSUM") as psC,
         ):
            for tt in range(NT_OWN):
                xn2 = wC.tile([128, D], F32, tag="p2_xn", name="p2_xn")
                _layernorm_tiles(nc, wC, eps_t, x2[:, tt, :], xn2)
                for dt in range(ND):
                    tp2 = psC.tile([128, 128], F32, tag="p2_tp", name="p2_tp")
                    nc.tensor.transpose(tp2, xn2[:, dt * 128 : (dt + 1) * 128], ident)
                    dst = xn2T[:, dt, tt * 128 : (tt + 1) * 128]
                    if dt % 2 == 0:
                        nc.vector.tensor_copy(out=dst, in_=tp2)
                    else:
                        nc.scalar.copy(out=dst, in_=tp2)

        # ---------------- Phase D: MLP + residual -> y ----------------
        if "D" in phases:
         with (
            tc.tile_pool(name="wD", bufs=3) as wD,
            tc.tile_pool(name="h1pool", bufs=1) as h1pool,
            tc.tile_pool(name="psD1", bufs=3, space="PSUM") as psD1,
            tc.tile_pool(name="psD2", bufs=1, space="PSUM") as psD2,
        ):
            for c2 in range(2):
                t0 = c2 * 512
                h1 = h1pool.tile([128, NH1, 512], BF16, tag="h1", name="h1", bufs=2)
                for ht in range(NH1):
                    w1s = wD.tile([128, ND, 128], BF16, tag="w1s", name="w1s")
                    nc.sync.dma_start(
                        out=w1s, in_=w1_ap[:, :, ht * 128 : (ht + 1) * 128]
                    )
                    psh = psD1.tile([128, 512], F32, tag="psh", name="psh")
                    for di in range(ND):
                        nc.tensor.matmul(
                            psh,
                            w1s[:, di, :],
                            xn2T[:, di, t0 : t0 + 512],
                            start=(di == 0),
                            stop=(di == ND - 1),
                        )
                    nc.scalar.activation(
                        out=h1[:, ht, :],
                        in_=psh,
                        func=AF.Gelu,
                        bias=b1t_sb[:, ht : ht + 1],
                        scale=1.0,
                    )
                for dch in range(2):
                    out_ps = [
                        psD2.tile([128, 512], F32, tag=f"o{i}", name=f"o{i}")
                        for i in range(4)
                    ]
                    for ts in range(4):
                        nc.tensor.matmul(
                            out_ps[ts],
                            ones_tok,
                            b2_sb[:, dch * 512 : (dch + 1) * 512],
                            start=True,
                            stop=False,
                        )
                    for ht in range(NH1):
                        w2s = wD.tile([128, 512], BF16, tag="w2s", name="w2s")
                        nc.sync.dma_start(
                            out=w2s, in_=w2_ap[:, ht, dch * 512 : (dch + 1) * 512]
                        )
                        for ts in range(4):
                            nc.tensor.matmul(
                                out_ps[ts],
                                h1[:, ht, ts * 128 : (ts + 1) * 128],
                                w2s,
                                start=False,
                                stop=(ht == NH1 - 1),
                            )
                    for ts in range(4):
                        tt = c2 * 4 + ts
                        yt = wD.tile([128, 512], F32, tag="yt", name="yt")
                        nc.vector.tensor_tensor(
                            out=yt,
                            in0=out_ps[ts],
                            in1=x2[:, tt, dch * 512 : (dch + 1) * 512],
                            op=OP.add,
                        )
                        nc.sync.dma_start(
                            out=y_d.ap()[
                                tt * 128 : (tt + 1) * 128,
                                dch * 512 : (dch + 1) * 512,
                            ],
                            in_=yt,
                        )

        p_xn2_cm.__exit__(None, None, None)
        gpersist_cm.__exit__(None, None, None)
        const_cm.__exit__(None, None, None)

    nc.compile()
    return nc


def _get_nc(has_qkv_bias: bool):
    key = ("v3", has_qkv_bias)
    if key not in _BUILD_CACHE:
        _BUILD_CACHE[key] = _build(has_qkv_bias)
    return _BUILD_CACHE[key]


# per-core inputs are sharded over the core mesh axis; everything else is
# broadcast once instead of being concatenated 8x (saves ~300MB of host->
# device transfer per call)
_SHARDED_INPUTS = {"x_ctx"}
_RUNNER_CACHE = {}


def _get_runner(has_qkv_bias: bool):
    key = has_qkv_bias
    if key in _RUNNER_CACHE:
        return _RUNNER_CACHE[key]

    import jax
    from jax.experimental.shard_map import shard_map
    from jax.sharding import Mesh, NamedSharding, PartitionSpec

    from concourse import bass2jax

    nc = _get_nc(has_qkv_bias)
    bass2jax.install_neuronx_cc_hook()
    partition_name = nc.partition_id_tensor.name if nc.partition_id_tensor else None

    in_names, out_names, out_avals, zero_outs = [], [], [], []
    for alloc in nc.m.functions[0].allocations:
        if not isinstance(alloc, mybir.MemoryLocationSet):
            continue
        name = alloc.memorylocations[0].name
        if alloc.kind == "ExternalInput":
            if name != partition_name:
                in_names.append(name)
        elif alloc.kind == "ExternalOutput":
            shape = tuple(alloc.tensor_shape)
            dtype = mybir.dt.np(alloc.dtype)
            out_names.append(name)
            out_avals.append(jax.core.ShapedArray(shape, dtype))
            zero_outs.append(np.zeros(shape, dtype))
    n_params = len(in_names)
    all_in_names = in_names + out_names
    if partition_name is not None:
        all_in_names.append(partition_name)

    def _body(*args):
        operands = list(args)
        if partition_name is not None:
            operands.append(bass2jax.partition_id_tensor())
        outs = bass2jax._bass_exec_p.bind(
            *operands,
            out_avals=tuple(out_avals),
            in_names=tuple(all_in_names),
            out_names=tuple(out_names),
            lowering_input_output_aliases=(),
            sim_require_finite=True,
            sim_require_nnan=True,
            nc=nc,
        )
        return tuple(outs)

    devices = jax.devices()[:NCORES]
    mesh = Mesh(np.asarray(devices), ("core",))
    core_spec = PartitionSpec("core")
    rep_spec = PartitionSpec()
    in_specs = tuple(
        core_spec if n in _SHARDED_INPUTS else rep_spec for n in in_names
    ) + (core_spec,) * len(out_names)
    out_specs = (core_spec,) * len(out_names)
    fn = jax.jit(
        shard_map(
            _body, mesh=mesh, in_specs=in_specs, out_specs=out_specs, check_rep=False
        ),
        keep_unused=True,
    )
    runner = {
        "fn": fn,
        "in_names": in_names,
        "out_names": out_names,
        "zero_outs": zero_outs,
        "mesh": mesh,
        "core_spec": core_spec,
        "rep_spec": rep_spec,
        "NamedSharding": NamedSharding,
        "jax": jax,
    }
    _RUNNER_CACHE[key] = runner
    return runner


def kernel(x, w_qkv, w_proj, b_proj, w1, b1, w2, b2, g1, be1, g2, be2, **_):
    x = np.ascontiguousarray(np.asarray(x, dtype=np.float32))
    w_qkv = np.asarray(w_qkv, dtype=np.float32)
    w_proj = np.asarray(w_proj, dtype=np.float32)
    b_proj = np.asarray(b_proj, dtype=np.float32)
    w1 = np.asarray(w1, dtype=np.float32)
    b1 = np.asarray(b1, dtype=np.float32)
    w2 = np.asarray(w2, dtype=np.float32)
    b2 = np.asarray(b2, dtype=np.float32)
    g1 = np.asarray(g1, dtype=np.float32)
    be1 = np.asarray(be1, dtype=np.float32)
    g2 = np.asarray(g2, dtype=np.float32)
    be2 = np.asarray(be2, dtype=np.float32)

    # fold LN affines into following matmuls (exact)
    wqkv_eff = w_qkv * g1[:, None]
    qkv_bias = be1 @ w_qkv
    bf = ml_dtypes.bfloat16
    inputs = {
        "wq": np.ascontiguousarray(wqkv_eff[:, :D].astype(bf)),
        "wk": np.ascontiguousarray(wqkv_eff[:, D : 2 * D].astype(bf)),
        "wv": np.ascontiguousarray(wqkv_eff[:, 2 * D :].astype(bf)),
        "wproj": np.ascontiguousarray(w_proj.astype(bf)),
        "w1": np.ascontiguousarray((w1 * g2[:, None]).astype(bf)),
        "w2": np.ascontiguousarray(w2.astype(bf)),
        "b1t": np.ascontiguousarray((b1 + be2 @ w1).reshape(NH1, 128).T),
        "b2": np.ascontiguousarray(b2[None, :]),
        "bp": np.ascontiguousarray(b_proj[None, :]),
    }
    has_qkv_bias = bool(np.any(qkv_bias != 0.0))
    if has_qkv_bias:
        inputs["bq"] = np.ascontiguousarray(qkv_bias[None, :D].astype(bf))
        inputs["bk"] = np.ascontiguousarray(qkv_bias[None, D : 2 * D].astype(bf))
        inputs["bv"] = np.ascontiguousarray(qkv_bias[None, 2 * D :].astype(bf))

    # per-core context: own 1024 tokens first, then the rest of its batch's
    # sequence (key order permutation -- exact for softmax attention)
    xf = x.reshape(NCORES, T_OWN, D)
    parts = []
    for c in range(NCORES):
        other = xf[c ^ 1]
        parts.append(np.concatenate([xf[c], other], axis=0))
    inputs["x_ctx"] = np.ascontiguousarray(np.stack(parts).reshape(NCORES * T_CTX, D))

    r = _get_runner(has_qkv_bias)
    jax = r["jax"]
    NamedSharding = r["NamedSharding"]
    dev_in = []
    for nname in r["in_names"]:
        spec = r["core_spec"] if nname in _SHARDED_INPUTS else r["rep_spec"]
        dev_in.append(
            jax.device_put(inputs[nname], NamedSharding(r["mesh"], spec))
        )
    for z in r["zero_outs"]:
        zc = np.zeros((NCORES * z.shape[0], *z.shape[1:]), z.dtype)
        dev_in.append(jax.device_put(zc, NamedSharding(r["mesh"], r["core_spec"])))
    outs = r["fn"](*dev_in)
    y = np.asarray(outs[r["out_names"].index("y")])
    global _LAST_RESULTS
    _LAST_RESULTS = outs
    return y.reshape(B, N, D)


# revision 25
# speedup vs baseline: 140.9914x; 95.7044x over previous
"""Transformer block (LN -> MHA -> residual -> LN -> MLP -> residual) on 8 trn2 cores.

Sharding: token-parallel. Core c owns 1024 tokens of batch b=c//2 (flattened
B*N = 8192 tokens / 8 cores). Attention needs full-sequence K/V, so each core
computes K/V for its batch's full 2048-token sequence (K/V projection is
duplicated across the 2 cores sharing a batch; ~12% extra FLOPs, zero
collectives). Key order within each core's context is permuted so its own
tokens come first -- softmax attention is permutation-invariant over keys, so
this is exact and lets all 8 cores run one identical program (SPMD, per-core
input maps).

Numerics: attention path matmuls in bf16 (inputs rounded, fp32 PSUM
accumulate), MLP/proj-adjacent big GEMMs in float32r (TF32-like, full fp32
storage, full PE rate). LayerNorm, softmax accumulation, residuals in fp32.
LN affine params are folded into the following weight matrices on the host
(exact); proj bias is folded into the residual input on the host.

Layout: activations feature-major ("transposed", [d, tokens]) for matmul
operands via PE-transpose after each LayerNorm; matmul outputs that feed
softmax/LN/residuals come out token-major. Softmax denominators ride along
the PV matmul as a ones-column appended to V (M=65); normalization uses a
K=1 ones-matmul to broadcast 1/denom across partitions.
"""

import numpy as np
import ml_dtypes

import concourse.bacc as bacc
import concourse.tile as tile
from concourse import mybir
from concourse.bass_utils import run_bass_kernel_spmd
from concourse.masks import make_identity

F32 = mybir.dt.float32
F32R = mybir.dt.float32r
BF16 = mybir.dt.bfloat16
AF = mybir.ActivationFunctionType
OP = mybir.AluOpType

D = 1024
H = 16
HD = 64
B = 4
N = 2048
DH = 4 * D
NCORES = 8
T_CTX = 2048
T_OWN = 1024
NT_CTX = T_CTX // 128  # 16
NT_OWN = T_OWN // 128  # 8
ND = D // 128  # 8
NH1 = DH // 128  # 32
EPS = 1e-5

_BUILD_CACHE = {}
_LAST_RESULTS = None  # outputs of the most recent run (for test harness)
_last_host_inputs = None  # prepared host input dict of the most recent run


def _layernorm_tiles(nc, work, const_eps, src_ap, out_tile):
    """LN stats + apply for one [128, D] token-major tile.

    Stats on DVE (bn_stats), apply on ACT: out = Identity(x * rstd - mu*rstd)
    with per-partition scale/bias keeps the big elementwise pass off the DVE.
    """
    stats = work.tile([128, 2, 6], F32, tag="ln_stats", name="ln_stats")
    xg = src_ap.rearrange("p (g d) -> p g d", g=2)
    for g in range(2):
        nc.vector.bn_stats(out=stats[:, g, :], in_=xg[:, g, :])
    mv = work.tile([128, 2], F32, tag="ln_mv", name="ln_mv")
    nc.vector.bn_aggr(out=mv, in_=stats)
    std = work.tile([128, 1], F32, tag="ln_std", name="ln_std")
    nc.scalar.activation(out=std, in_=mv[:, 1:2], func=AF.Sqrt, bias=const_eps, scale=1.0)
    rstd = work.tile([128, 1], F32, tag="ln_rstd", name="ln_rstd")
    nc.vector.reciprocal(out=rstd, in_=std)
    nc.vector.tensor_scalar(
        out=out_tile,
        in0=src_ap,
        scalar1=mv[:, 0:1],
        scalar2=rstd,
        op0=OP.subtract,
        op1=OP.mult,
    )


def _build(has_qkv_bias: bool, phases: str = "0ABCD"):
    nc = bacc.Bacc("TRN2", target_bir_lowering=False, debug=False)

    x_ctx = nc.dram_tensor("x_ctx", [T_CTX, D], F32, kind="ExternalInput")
    wq_d = nc.dram_tensor("wq", [D, D], BF16, kind="ExternalInput")
    wk_d = nc.dram_tensor("wk", [D, D], BF16, kind="ExternalInput")
    wv_d = nc.dram_tensor("wv", [D, D], BF16, kind="ExternalInput")
    wp_d = nc.dram_tensor("wproj", [D, D], BF16, kind="ExternalInput")
    w1_d = nc.dram_tensor("w1", [D, DH], BF16, kind="ExternalInput")
    w2_d = nc.dram_tensor("w2", [DH, D], BF16, kind="ExternalInput")
    b1t_d = nc.dram_tensor("b1t", [128, NH1], F32, kind="ExternalInput")
    b2_d = nc.dram_tensor("b2", [1, D], F32R, kind="ExternalInput")
    bp_d = nc.dram_tensor("bp", [1, D], F32R, kind="ExternalInput")
    if has_qkv_bias:
        bq_d = nc.dram_tensor("bq", [1, D], BF16, kind="ExternalInput")
        bk_d = nc.dram_tensor("bk", [1, D], BF16, kind="ExternalInput")
        bv_d = nc.dram_tensor("bv", [1, D], BF16, kind="ExternalInput")
    y_d = nc.dram_tensor("y", [T_OWN, D], F32, kind="ExternalOutput")

    wq_t = wq_d.ap().rearrange("(t p) o -> p t o", p=128)
    wk_t = wk_d.ap().rearrange("(t p) o -> p t o", p=128)
    wv_t = wv_d.ap().rearrange("(t p) o -> p t o", p=128)
    wp_t = wp_d.ap().rearrange("(t p) o -> p t o", p=128)
    w1_ap = w1_d.ap().rearrange("(t p) o -> p t o", p=128)
    w2_ap = w2_d.ap().rearrange("(t p) o -> p t o", p=128)

    with tile.TileContext(nc) as tc:
        const_cm = tc.tile_pool(name="const", bufs=1)
        const = const_cm.__enter__()
        eps_t = const.tile([128, 1], F32)
        nc.vector.memset(eps_t, EPS)
        ident = const.tile([128, 128], F32)
        make_identity(nc, ident)
        ones_f = const.tile([1, 128], F32)  # memset can't write f32r directly
        nc.vector.memset(ones_f, 1.0)
        ones_h = const.tile([1, HD], F32R)  # lhsT for 1/denom broadcast
        nc.vector.tensor_copy(out=ones_h, in_=ones_f[:, :HD])
        ones_tok = const.tile([1, 128], F32R)  # lhsT for fc2 bias init
        nc.vector.tensor_copy(out=ones_tok, in_=ones_f)
        b1t_sb = const.tile([128, NH1], F32)
        nc.sync.dma_start(out=b1t_sb, in_=b1t_d.ap())
        b2_sb = const.tile([1, D], F32R)
        nc.sync.dma_start(out=b2_sb, in_=b2_d.ap())
        bp_sb = const.tile([1, D], F32R)
        nc.sync.dma_start(out=bp_sb, in_=bp_d.ap())
        if has_qkv_bias:
            ones_512b = const.tile([1, 512], BF16)
            nc.vector.memset(ones_512b, 1.0)
            ones_tokb = const.tile([1, 128], BF16)
            nc.vector.memset(ones_tokb, 1.0)
            bq_sb = const.tile([1, D], BF16)
            nc.sync.dma_start(out=bq_sb, in_=bq_d.ap())
            bk_sb = const.tile([1, D], BF16)
            nc.sync.dma_start(out=bk_sb, in_=bk_d.ap())
            bv_sb = const.tile([1, D], BF16)
            nc.sync.dma_start(out=bv_sb, in_=bv_d.ap())

        gpersist_cm = tc.tile_pool(name="gpersist", bufs=1)
        gpersist = gpersist_cm.__enter__()
        x2 = gpersist.tile([128, NT_OWN, D], F32)  # 32KB/part, phases B..E

        # xn1T + attnT live phases 0..B (released together, LIFO-nested)
        p_big1_cm = tc.tile_pool(name="p_big1", bufs=1)
        p_big1 = p_big1_cm.__enter__()
        xn1T = p_big1.tile([128, ND, T_CTX], BF16)  # 32KB/part
        attnT = p_big1.tile([128, ND, T_OWN], BF16)  # 16KB/part

        # ---------------- Phase 0: LN1 + transpose -> xn1T ----------------
        if "0" in phases:
         with (
            tc.tile_pool(name="w0", bufs=3) as w0,
            tc.tile_pool(name="ps0", bufs=4, space="PSUM") as ps0,
        ):
            for t4 in range(NT_CTX // 4):
              x4 = w0.tile([128, 4, D], F32, tag="p0_x", name="p0_x", bufs=2)
              nc.sync.dma_start(
                  out=x4,
                  in_=x_ctx.ap()[t4 * 512 : (t4 + 1) * 512, :].rearrange(
                      "(f p) d -> p f d", p=128
                  ),
              )
              for ti in range(4):
                t = t4 * 4 + ti
                xt = x4[:, ti, :]
                xn = w0.tile([128, D], F32, tag="p0_xn", name="p0_xn")
                _layernorm_tiles(nc, w0, eps_t, xt, xn)
                for dt in range(ND):
                    tp = ps0.tile([128, 128], F32, tag="p0_tp", name="p0_tp")
                    nc.tensor.transpose(tp, xn[:, dt * 128 : (dt + 1) * 128], ident)
                    dst = xn1T[:, dt, t * 128 : (t + 1) * 128]
                    if dt % 2 == 0:
                        nc.vector.tensor_copy(out=dst, in_=tp)
                    else:
                        nc.scalar.copy(out=dst, in_=tp)

        # ---------------- Phase A: attention -> attnT ----------------
        if "A" in phases:
         with (
            tc.tile_pool(name="wA", bufs=1) as wA,
            tc.tile_pool(name="psA", bufs=1, space="PSUM") as psA,
        ):
            vt_tiles = [None] * NT_CTX
            for p in range(H // 2):
                g = p // 4
                if p % 4 == 0:
                    wv_g = wA.tile([128, ND, 512], BF16, tag="wv_g", name="wv_g")
                    nc.sync.dma_start(out=wv_g, in_=wv_t[:, :, g * 512 : (g + 1) * 512])
                    for kt_i in range(NT_CTX):
                        psv = psA.tile([128, 512], F32, tag="mm512", name="psv", bufs=2)
                        if has_qkv_bias:
                            nc.tensor.matmul(
                                psv,
                                ones_tokb,
                                bv_sb[:, g * 512 : (g + 1) * 512],
                                start=True,
                                stop=False,
                            )
                        for di in range(ND):
                            nc.tensor.matmul(
                                psv,
                                xn1T[:, di, kt_i * 128 : (kt_i + 1) * 128],
                                wv_g[:, di, :],
                                start=(di == 0 and not has_qkv_bias),
                                stop=(di == ND - 1),
                            )
                        vt = wA.tile(
                            [128, 8, 65], BF16, tag=f"vt{kt_i}", name=f"vt{kt_i}"
                        )
                        nc.vector.memset(vt[:, :, 64:65], 1.0)
                        nc.vector.tensor_copy(
                            out=vt[:, :, 0:64],
                            in_=psv.rearrange("p (h d) -> p h d", h=8),
                        )
                        vt_tiles[kt_i] = vt

                wk_p = wA.tile([128, ND, 128], BF16, tag="wk_p", name="wk_p", bufs=2)
                nc.sync.dma_start(out=wk_p, in_=wk_t[:, :, p * 128 : (p + 1) * 128])
                wq_p = wA.tile([128, ND, 128], BF16, tag="wq_p", name="wq_p", bufs=2)
                nc.sync.dma_start(out=wq_p, in_=wq_t[:, :, p * 128 : (p + 1) * 128])

                ktp = wA.tile([128, T_CTX], BF16, tag="ktp", name="ktp", bufs=2)
                for ch in range(T_CTX // 512):
                    psk = psA.tile([128, 512], F32, tag="mm512", name="psk", bufs=2)
                    if has_qkv_bias:
                        nc.tensor.matmul(
                            psk,
                            bk_sb[:, p * 128 : (p + 1) * 128],
                            ones_512b,
                            start=True,
                            stop=False,
                        )
                    for di in range(ND):
                        nc.tensor.matmul(
                            psk,
                            wk_p[:, di, :],
                            xn1T[:, di, ch * 512 : (ch + 1) * 512],
                            start=(di == 0 and not has_qkv_bias),
                            stop=(di == ND - 1),
                        )
                    nc.vector.tensor_copy(out=ktp[:, ch * 512 : (ch + 1) * 512], in_=psk)

                qtp = wA.tile([128, T_OWN], BF16, tag="qtp", name="qtp", bufs=2)
                for ch in range(T_OWN // 512):
                    psq = psA.tile([128, 512], F32, tag="mm512", name="psq", bufs=2)
                    if has_qkv_bias:
                        nc.tensor.matmul(
                            psq,
                            bq_sb[:, p * 128 : (p + 1) * 128],
                            ones_512b,
                            start=True,
                            stop=False,
                        )
                    for di in range(ND):
                        nc.tensor.matmul(
                            psq,
                            wq_p[:, di, :],
                            xn1T[:, di, ch * 512 : (ch + 1) * 512],
                            start=(di == 0 and not has_qkv_bias),
                            stop=(di == ND - 1),
                        )
                    nc.vector.tensor_copy(out=qtp[:, ch * 512 : (ch + 1) * 512], in_=psq)

                for qc in range(T_OWN // 512 if "S" not in phases else 0):
                    q0 = qc * 512
                    ov = [
                        psA.tile([65, 512], F32, tag=f"ov{h}", name=f"ov{h}", bufs=1)
                        for h in range(2)
                    ]

                    def _s_exp(kt_i):
                        st = psA.tile(
                            [128, 2, 512], F32, tag="stpair", name="st", bufs=2
                        )
                        for h in range(2):
                            nc.tensor.matmul(
                                st[:, h, :],
                                ktp[h * 64 : (h + 1) * 64, kt_i * 128 : (kt_i + 1) * 128],
                                qtp[h * 64 : (h + 1) * 64, q0 : q0 + 512],
                                start=True,
                                stop=True,
                            )
                        ptm = wA.tile(
                            [128, 2, 512], BF16, tag="ptm", name="ptm", bufs=4
                        )
                        nc.scalar.activation(out=ptm, in_=st, func=AF.Exp, scale=0.125)
                        return [ptm[:, 0, :], ptm[:, 1, :]]

                    # software-pipeline: keep one S/exp in flight ahead of PV
                    # so the PE never stalls on the ACT exp of the current tile
                    pts_prev = _s_exp(0)
                    for kt_i in range(NT_CTX):
                        pts_next = _s_exp(kt_i + 1) if kt_i + 1 < NT_CTX else None
                        for h in range(2):
                            nc.tensor.matmul(
                                ov[h],
                                vt_tiles[kt_i][:, 2 * (p % 4) + h, :],
                                pts_prev[h],
                                start=(kt_i == 0),
                                stop=(kt_i == NT_CTX - 1),
                            )
                        pts_prev = pts_next
                    for h in range(2):
                        ovsb = wA.tile([65, 512], F32, tag="ovsb", name="ovsb", bufs=2)
                        nc.vector.tensor_copy(out=ovsb, in_=ov[h])
                        rec = wA.tile([1, 512], F32R, tag="rec", name="rec", bufs=2)
                        with nc.allow_low_precision(reason="f32r softmax denom"):
                            nc.vector.reciprocal(out=rec, in_=ovsb[64:65, :])
                        bc = psA.tile([64, 512], F32, tag="mm512", name="bc", bufs=2)
                        nc.tensor.matmul(bc, ones_h, rec, start=True, stop=True)
                        nc.vector.tensor_tensor(
                            out=attnT[h * 64 : (h + 1) * 64, p, q0 : q0 + 512],
                            in0=ovsb[0:64, :],
                            in1=bc,
                            op=OP.mult,
                        )


        # ---------------- Phase B: proj + residual -> x2 ----------------
        if "B" in phases:
         with (
            tc.tile_pool(name="wB", bufs=3) as wB,
            tc.tile_pool(name="psB", bufs=4, space="PSUM") as psB,
        ):
            wp_sb = wB.tile([128, ND, D], BF16, tag="wp_sb", name="wp_sb", bufs=1)
            nc.sync.dma_start(out=wp_sb, in_=wp_t)
            for tt4 in range(NT_OWN // 4):
              xr4 = wB.tile([128, 4, D], F32, tag="xr_t", name="xr_t", bufs=2)
              nc.sync.dma_start(
                  out=xr4,
                  in_=x_ctx.ap()[tt4 * 512 : (tt4 + 1) * 512, :].rearrange(
                      "(f p) d -> p f d", p=128
                  ),
              )
              for ti in range(4):
                tt = tt4 * 4 + ti
                xr_t = xr4[:, ti, :]
                for ch in range(2):
                    psb = psB.tile([128, 512], F32, tag="psb", name="psb")
                    nc.tensor.matmul(
                        psb,
                        ones_tok,
                        bp_sb[:, ch * 512 : (ch + 1) * 512],
                        start=True,
                        stop=False,
                    )
                    for di in range(ND):
                        nc.tensor.matmul(
                            psb,
                            attnT[:, di, tt * 128 : (tt + 1) * 128],
                            wp_sb[:, di, ch * 512 : (ch + 1) * 512],
                            start=False,
                            stop=(di == ND - 1),
                        )
                    nc.vector.tensor_tensor(
                        out=x2[:, tt, ch * 512 : (ch + 1) * 512],
                        in0=psb,
                        in1=xr_t[:, ch * 512 : (ch + 1) * 512],
                        op=OP.add,
                    )

        p_big1_cm.__exit__(None, None, None)  # free xn1T + attnT

        # ---------------- Phase C: LN2 + transpose -> xn2T ----------------
        p_xn2_cm = tc.tile_pool(name="p_xn2", bufs=1)
        p_xn2 = p_xn2_cm.__enter__()
        xn2T = p_xn2.tile([128, ND, T_OWN], BF16)  # 16KB/part

        if "C" in phases:
         with (
            tc.tile_pool(name="wC", bufs=3) as wC,
            tc.tile_pool(name="psC", bufs=4, space="PSUM") as psC,
         ):
            for tt in range(NT_OWN):
                xn2 = wC.tile([128, D], F32, tag="p2_xn", name="p2_xn")
                _layernorm_tiles(nc, wC, eps_t, x2[:, tt, :], xn2)
                for dt in range(ND):
                    tp2 = psC.tile([128, 128], F32, tag="p2_tp", name="p2_tp")
                    nc.tensor.transpose(tp2, xn2[:, dt * 128 : (dt + 1) * 128], ident)
                    dst = xn2T[:, dt, tt * 128 : (tt + 1) * 128]
                    if dt % 2 == 0:
                        nc.vector.tensor_copy(out=dst, in_=tp2)
                    else:
                        nc.scalar.copy(out=dst, in_=tp2)

        # ---------------- Phase D: MLP + residual -> y ----------------
        if "D" in phases:
         with (
            tc.tile_pool(name="wD", bufs=3) as wD,
            tc.tile_pool(name="h1pool", bufs=1) as h1pool,
            tc.tile_pool(name="psD1", bufs=3, space="PSUM") as psD1,
            tc.tile_pool(name="psD2", bufs=1, space="PSUM") as psD2,
        ):
            for c2 in range(2):
                t0 = c2 * 512
                h1 = h1pool.tile([128, NH1, 512], BF16, tag="h1", name="h1", bufs=2)
                for ht4 in range(NH1 // 4):
                    w1s = wD.tile(
                        [128, ND, 4, 128], BF16, tag="w1s", name="w1s", bufs=2
                    )
                    nc.sync.dma_start(
                        out=w1s,
                        in_=w1_ap[:, :, ht4 * 512 : (ht4 + 1) * 512].rearrange(
                            "p t (f o) -> p t f o", f=4
                        ),
                    )
                    for hi in range(4):
                        ht = ht4 * 4 + hi
                        psh = psD1.tile([128, 512], F32, tag="psh", name="psh")
                        for di in range(ND):
                            nc.tensor.matmul(
                                psh,
                                w1s[:, di, hi, :],
                                xn2T[:, di, t0 : t0 + 512],
                                start=(di == 0),
                                stop=(di == ND - 1),
                            )
                        nc.scalar.activation(
                            out=h1[:, ht, :],
                            in_=psh,
                            func=AF.Gelu,
                            bias=b1t_sb[:, ht : ht + 1],
                            scale=1.0,
                        )
                for dch in range(2):
                    out_ps = [
                        psD2.tile([128, 512], F32, tag=f"o{i}", name=f"o{i}")
                        for i in range(4)
                    ]
                    for ts in range(4):
                        nc.tensor.matmul(
                            out_ps[ts],
                            ones_tok,
                            b2_sb[:, dch * 512 : (dch + 1) * 512],
                            start=True,
                            stop=False,
                        )
                    for ht4 in range(NH1 // 4):
                        w2s = wD.tile(
                            [128, 4, 512], BF16, tag="w2s", name="w2s", bufs=3
                        )
                        nc.sync.dma_start(
                            out=w2s,
                            in_=w2_ap[:, ht4 * 4 : (ht4 + 1) * 4, dch * 512 : (dch + 1) * 512],
                        )
                        for hi in range(4):
                            ht = ht4 * 4 + hi
                            for ts in range(4):
                                nc.tensor.matmul(
                                    out_ps[ts],
                                    h1[:, ht, ts * 128 : (ts + 1) * 128],
                                    w2s[:, hi, :],
                                    start=False,
                                    stop=(ht == NH1 - 1),
                                )
                    for ts in range(4):
                        tt = c2 * 4 + ts
                        yt = wD.tile([128, 512], F32, tag="yt", name="yt")
                        nc.vector.tensor_tensor(
                            out=yt,
                            in0=out_ps[ts],
                            in1=x2[:, tt, dch * 512 : (dch + 1) * 512],
                            op=OP.add,
                        )
                        nc.sync.dma_start(
                            out=y_d.ap()[
                                tt * 128 : (tt + 1) * 128,
                                dch * 512 : (dch + 1) * 512,
                            ],
                            in_=yt,
                        )

        p_xn2_cm.__exit__(None, None, None)
        gpersist_cm.__exit__(None, None, None)
        const_cm.__exit__(None, None, None)

    nc.compile()
    return nc


def _get_nc(has_qkv_bias: bool):
    key = ("v3", has_qkv_bias)
    if key not in _BUILD_CACHE:
        _BUILD_CACHE[key] = _build(has_qkv_bias)
    return _BUILD_CACHE[key]


# per-core inputs are sharded over the core mesh axis; everything else is
# broadcast once instead of being concatenated 8x (saves ~300MB of host->
# device transfer per call)
_SHARDED_INPUTS = {"x_ctx"}
_RUNNER_CACHE = {}


def _get_runner(has_qkv_bias: bool):
    key = has_qkv_bias
    if key in _RUNNER_CACHE:
        return _RUNNER_CACHE[key]

    import jax
    from jax.experimental.shard_map import shard_map
    from jax.sharding import Mesh, NamedSharding, PartitionSpec

    from concourse import bass2jax

    nc = _get_nc(has_qkv_bias)
    bass2jax.install_neuronx_cc_hook()
    partition_name = nc.partition_id_tensor.name if nc.partition_id_tensor else None

    in_names, out_names, out_avals, zero_outs = [], [], [], []
    for alloc in nc.m.functions[0].allocations:
        if not isinstance(alloc, mybir.MemoryLocationSet):
            continue
        name = alloc.memorylocations[0].name
        if alloc.kind == "ExternalInput":
            if name != partition_name:
                in_names.append(name)
        elif alloc.kind == "ExternalOutput":
            shape = tuple(alloc.tensor_shape)
            dtype = mybir.dt.np(alloc.dtype)
            out_names.append(name)
            out_avals.append(jax.core.ShapedArray(shape, dtype))
            zero_outs.append(np.zeros(shape, dtype))
    n_params = len(in_names)
    all_in_names = in_names + out_names
    if partition_name is not None:
        all_in_names.append(partition_name)

    def _body(*args):
        operands = list(args)
        if partition_name is not None:
            operands.append(bass2jax.partition_id_tensor())
        outs = bass2jax._bass_exec_p.bind(
            *operands,
            out_avals=tuple(out_avals),
            in_names=tuple(all_in_names),
            out_names=tuple(out_names),
            lowering_input_output_aliases=(),
            sim_require_finite=True,
            sim_require_nnan=True,
            nc=nc,
        )
        return tuple(outs)

    devices = jax.devices()[:NCORES]
    mesh = Mesh(np.asarray(devices), ("core",))
    core_spec = PartitionSpec("core")
    rep_spec = PartitionSpec()
    in_specs = tuple(
        core_spec if n in _SHARDED_INPUTS else rep_spec for n in in_names
    ) + (core_spec,) * len(out_names)
    out_specs = (core_spec,) * len(out_names)
    fn = jax.jit(
        shard_map(
            _body, mesh=mesh, in_specs=in_specs, out_specs=out_specs, check_rep=False
        ),
        keep_unused=True,
    )
    dev_zeros = [
        jax.device_put(
            np.zeros((NCORES * z.shape[0], *z.shape[1:]), z.dtype),
            NamedSharding(mesh, core_spec),
        )
        for z in zero_outs
    ]
    runner = {
        "fn": fn,
        "in_names": in_names,
        "out_names": out_names,
        "zero_outs": zero_outs,
        "dev_zeros": dev_zeros,
        "mesh": mesh,
        "core_spec": core_spec,
        "rep_spec": rep_spec,
        "NamedSharding": NamedSharding,
        "jax": jax,
    }
    _RUNNER_CACHE[key] = runner
    return runner


def kernel(x, w_qkv, w_proj, b_proj, w1, b1, w2, b2, g1, be1, g2, be2, **_):
    x = np.ascontiguousarray(np.asarray(x, dtype=np.float32))
    w_qkv = np.asarray(w_qkv, dtype=np.float32)
    w_proj = np.asarray(w_proj, dtype=np.float32)
    b_proj = np.asarray(b_proj, dtype=np.float32)
    w1 = np.asarray(w1, dtype=np.float32)
    b1 = np.asarray(b1, dtype=np.float32)
    w2 = np.asarray(w2, dtype=np.float32)
    b2 = np.asarray(b2, dtype=np.float32)
    g1 = np.asarray(g1, dtype=np.float32)
    be1 = np.asarray(be1, dtype=np.float32)
    g2 = np.asarray(g2, dtype=np.float32)
    be2 = np.asarray(be2, dtype=np.float32)

    # fold LN affines into following matmuls (exact)
    wqkv_eff = w_qkv * g1[:, None]
    qkv_bias = be1 @ w_qkv
    bf = ml_dtypes.bfloat16
    inputs = {
        "wq": np.ascontiguousarray(wqkv_eff[:, :D].astype(bf)),
        "wk": np.ascontiguousarray(wqkv_eff[:, D : 2 * D].astype(bf)),
        "wv": np.ascontiguousarray(wqkv_eff[:, 2 * D :].astype(bf)),
        "wproj": np.ascontiguousarray(w_proj.astype(bf)),
        "w1": np.ascontiguousarray((w1 * g2[:, None]).astype(bf)),
        "w2": np.ascontiguousarray(w2.astype(bf)),
        "b1t": np.ascontiguousarray((b1 + be2 @ w1).reshape(NH1, 128).T),
        "b2": np.ascontiguousarray(b2[None, :]),
        "bp": np.ascontiguousarray(b_proj[None, :]),
    }
    has_qkv_bias = bool(np.any(qkv_bias != 0.0))
    if has_qkv_bias:
        inputs["bq"] = np.ascontiguousarray(qkv_bias[None, :D].astype(bf))
        inputs["bk"] = np.ascontiguousarray(qkv_bias[None, D : 2 * D].astype(bf))
        inputs["bv"] = np.ascontiguousarray(qkv_bias[None, 2 * D :].astype(bf))

    # per-core context: own 1024 tokens first, then the rest of its batch's
    # sequence (key order permutation -- exact for softmax attention)
    xf = x.reshape(NCORES, T_OWN, D)
    parts = []
    for c in range(NCORES):
        other = xf[c ^ 1]
        parts.append(np.concatenate([xf[c], other], axis=0))
    inputs["x_ctx"] = np.ascontiguousarray(np.stack(parts).reshape(NCORES * T_CTX, D))

    global _last_host_inputs
    _last_host_inputs = inputs

    r = _get_runner(has_qkv_bias)
    jax = r["jax"]
    NamedSharding = r["NamedSharding"]
    dev_in = []
    for nname in r["in_names"]:
        spec = r["core_spec"] if nname in _SHARDED_INPUTS else r["rep_spec"]
        dev_in.append(
            jax.device_put(inputs[nname], NamedSharding(r["mesh"], spec))
        )
    dev_in.extend(r["dev_zeros"])
    outs = r["fn"](*dev_in)
    y = np.asarray(outs[r["out_names"].index("y")])
    global _LAST_RESULTS
    _LAST_RESULTS = outs
    return y.reshape(B, N, D)


# revision 30
# speedup vs baseline: 176.5004x; 1.2519x over previous
"""Transformer block (LN -> MHA -> residual -> LN -> MLP -> residual) on 8 trn2 cores.

Sharding: token-parallel. Core c owns 1024 tokens of batch b=c//2 (flattened
B*N = 8192 tokens / 8 cores). Attention needs full-sequence K/V, so each core
computes K/V for its batch's full 2048-token sequence (K/V projection is
duplicated across the 2 cores sharing a batch; ~12% extra FLOPs, zero
collectives). Key order within each core's context is permuted so its own
tokens come first -- softmax attention is permutation-invariant over keys, so
this is exact and lets all 8 cores run one identical program (SPMD, per-core
input maps).

Numerics: all big GEMMs in bf16 operands with fp32 PSUM accumulation (full
PE rate, ~1e-3 headroom at the output); LayerNorm statistics, softmax
accumulation/normalization, biases and residual adds in fp32 (softmax
reciprocal in float32r). LN affine params are folded into the following
weight matrices on the host (exact); proj/fc2 biases enter via a K=1
ones-outer-product matmul that initializes the output PSUM accumulators.

Layout: activations feature-major ("transposed", [d, tokens]) for matmul
operands via PE-transpose after each LayerNorm; matmul outputs that feed
softmax/LN/residuals come out token-major. Softmax denominators ride along
the PV matmul as a ones-column appended to V (M=65); normalization uses a
K=1 ones-matmul to broadcast 1/denom across partitions.
"""

import numpy as np
import ml_dtypes

import concourse.bacc as bacc
import concourse.tile as tile
from concourse import mybir
from concourse.masks import make_identity

F32 = mybir.dt.float32
F32R = mybir.dt.float32r
BF16 = mybir.dt.bfloat16
AF = mybir.ActivationFunctionType
OP = mybir.AluOpType

D = 1024
H = 16
HD = 64
B = 4
N = 2048
DH = 4 * D
NCORES = 8
T_CTX = 2048
T_OWN = 1024
NT_CTX = T_CTX // 128  # 16
NT_OWN = T_OWN // 128  # 8
ND = D // 128  # 8
NH1 = DH // 128  # 32
EPS = 1e-5

_BUILD_CACHE = {}
_LAST_RESULTS = None  # outputs of the most recent run (for test harness)
_last_host_inputs = None  # prepared host input dict of the most recent run


def _layernorm_tiles(nc, work, const_eps, src_ap, out_tile):
    """LN stats + apply for one [128, D] token-major tile.

    Stats on DVE (bn_stats), apply on ACT: out = Identity(x * rstd - mu*rstd)
    with per-partition scale/bias keeps the big elementwise pass off the DVE.
    """
    stats = work.tile([128, 2, 6], F32, tag="ln_stats", name="ln_stats")
    xg = src_ap.rearrange("p (g d) -> p g d", g=2)
    for g in range(2):
        nc.vector.bn_stats(out=stats[:, g, :], in_=xg[:, g, :])
    mv = work.tile([128, 2], F32, tag="ln_mv", name="ln_mv")
    nc.vector.bn_aggr(out=mv, in_=stats)
    std = work.tile([128, 1], F32, tag="ln_std", name="ln_std")
    nc.scalar.activation(out=std, in_=mv[:, 1:2], func=AF.Sqrt, bias=const_eps, scale=1.0)
    rstd = work.tile([128, 1], F32, tag="ln_rstd", name="ln_rstd")
    nc.vector.reciprocal(out=rstd, in_=std)
    nc.vector.tensor_scalar(
        out=out_tile,
        in0=src_ap,
        scalar1=mv[:, 0:1],
        scalar2=rstd,
        op0=OP.subtract,
        op1=OP.mult,
    )


def _build(has_qkv_bias: bool, phases: str = "0ABCD"):
    nc = bacc.Bacc("TRN2", target_bir_lowering=False, debug=False)

    x_ctx = nc.dram_tensor("x_ctx", [T_CTX, D], F32, kind="ExternalInput")
    wq_d = nc.dram_tensor("wq", [D, D], BF16, kind="ExternalInput")
    wk_d = nc.dram_tensor("wk", [D, D], BF16, kind="ExternalInput")
    wv_d = nc.dram_tensor("wv", [D, D], BF16, kind="ExternalInput")
    wp_d = nc.dram_tensor("wproj", [D, D], BF16, kind="ExternalInput")
    w1_d = nc.dram_tensor("w1", [D, DH], BF16, kind="ExternalInput")
    w2_d = nc.dram_tensor("w2", [DH, D], BF16, kind="ExternalInput")
    b1t_d = nc.dram_tensor("b1t", [128, NH1], F32, kind="ExternalInput")
    b2_d = nc.dram_tensor("b2", [1, D], F32R, kind="ExternalInput")
    bp_d = nc.dram_tensor("bp", [1, D], F32R, kind="ExternalInput")
    if has_qkv_bias:
        bq_d = nc.dram_tensor("bq", [1, D], BF16, kind="ExternalInput")
        bk_d = nc.dram_tensor("bk", [1, D], BF16, kind="ExternalInput")
        bv_d = nc.dram_tensor("bv", [1, D], BF16, kind="ExternalInput")
    y_d = nc.dram_tensor("y", [T_OWN, D], F32, kind="ExternalOutput")

    wq_t = wq_d.ap().rearrange("(t p) o -> p t o", p=128)
    wk_t = wk_d.ap().rearrange("(t p) o -> p t o", p=128)
    wv_t = wv_d.ap().rearrange("(t p) o -> p t o", p=128)
    wp_t = wp_d.ap().rearrange("(t p) o -> p t o", p=128)
    w1_ap = w1_d.ap().rearrange("(t p) o -> p t o", p=128)
    w2_ap = w2_d.ap().rearrange("(t p) o -> p t o", p=128)

    with tile.TileContext(nc) as tc:
        const_cm = tc.tile_pool(name="const", bufs=1)
        const = const_cm.__enter__()
        eps_t = const.tile([128, 1], F32)
        nc.vector.memset(eps_t, EPS)
        ident = const.tile([128, 128], F32)
        make_identity(nc, ident)
        ident_b = const.tile([128, 128], BF16)
        make_identity(nc, ident_b)
        ones_f = const.tile([1, 128], F32)  # memset can't write f32r directly
        nc.vector.memset(ones_f, 1.0)
        ones_h = const.tile([1, HD], F32R)  # lhsT for 1/denom broadcast
        nc.vector.tensor_copy(out=ones_h, in_=ones_f[:, :HD])
        ones_tok = const.tile([1, 128], F32R)  # lhsT for fc2 bias init
        nc.vector.tensor_copy(out=ones_tok, in_=ones_f)
        b1t_sb = const.tile([128, NH1], F32)
        nc.sync.dma_start(out=b1t_sb, in_=b1t_d.ap())
        b2_sb = const.tile([1, D], F32R)
        nc.sync.dma_start(out=b2_sb, in_=b2_d.ap())
        bp_sb = const.tile([1, D], F32R)
        nc.sync.dma_start(out=bp_sb, in_=bp_d.ap())
        if has_qkv_bias:
            ones_512b = const.tile([1, 512], BF16)
            nc.vector.memset(ones_512b, 1.0)
            ones_tokb = const.tile([1, 128], BF16)
            nc.vector.memset(ones_tokb, 1.0)
            bq_sb = const.tile([1, D], BF16)
            nc.sync.dma_start(out=bq_sb, in_=bq_d.ap())
            bk_sb = const.tile([1, D], BF16)
            nc.sync.dma_start(out=bk_sb, in_=bk_d.ap())
            bv_sb = const.tile([1, D], BF16)
            nc.sync.dma_start(out=bv_sb, in_=bv_d.ap())

        gpersist_cm = tc.tile_pool(name="gpersist", bufs=1)
        gpersist = gpersist_cm.__enter__()
        x2 = gpersist.tile([128, NT_OWN, D], F32)  # 32KB/part, phases B..E

        # xn2T lives phases B..D; created before p_big1 so pool exits stay LIFO
        p_xn2_cm = tc.tile_pool(name="p_xn2", bufs=1)
        p_xn2 = p_xn2_cm.__enter__()
        xn2T = p_xn2.tile([128, ND, T_OWN], BF16)  # 16KB/part

        # xn1T + attnT live phases 0..B (released together, LIFO-nested)
        p_big1_cm = tc.tile_pool(name="p_big1", bufs=1)
        p_big1 = p_big1_cm.__enter__()
        xn1T = p_big1.tile([128, ND, T_CTX], BF16)  # 32KB/part
        attnT = p_big1.tile([128, ND, T_OWN], BF16)  # 16KB/part

        # ---------------- Phase 0: LN1 + transpose -> xn1T ----------------
        if "0" in phases:
         with (
            tc.tile_pool(name="w0", bufs=3) as w0,
            tc.tile_pool(name="ps0", bufs=4, space="PSUM") as ps0,
        ):
            for t4 in range(NT_CTX // 4):
              x4 = w0.tile([128, 4, D], F32, tag="p0_x", name="p0_x", bufs=2)
              nc.sync.dma_start(
                  out=x4,
                  in_=x_ctx.ap()[t4 * 512 : (t4 + 1) * 512, :].rearrange(
                      "(f p) d -> p f d", p=128
                  ),
              )
              for ti in range(4):
                t = t4 * 4 + ti
                xt = x4[:, ti, :]
                xn = w0.tile([128, D], BF16, tag="p0_xn", name="p0_xn")
                _layernorm_tiles(nc, w0, eps_t, xt, xn)
                for dt in range(ND):
                    tp = ps0.tile([128, 128], BF16, tag="p0_tp", name="p0_tp")
                    nc.tensor.transpose(tp, xn[:, dt * 128 : (dt + 1) * 128], ident_b)
                    dst = xn1T[:, dt, t * 128 : (t + 1) * 128]
                    if dt % 2 == 0:
                        nc.vector.tensor_copy(out=dst, in_=tp)
                    else:
                        nc.scalar.copy(out=dst, in_=tp)

        # ---------------- Phase A: attention -> attnT ----------------
        if "A" in phases:
         with (
            tc.tile_pool(name="wA", bufs=1) as wA,
            tc.tile_pool(name="psA", bufs=1, space="PSUM") as psA,
        ):
            vt_tiles = [None] * NT_CTX
            for p in range(H // 2):
                g = p // 4
                if p % 4 == 0:
                    wv_g = wA.tile([128, ND, 512], BF16, tag="wv_g", name="wv_g")
                    nc.sync.dma_start(out=wv_g, in_=wv_t[:, :, g * 512 : (g + 1) * 512])
                    for kt_i in range(NT_CTX):
                        psv = psA.tile([128, 512], F32, tag="mm512", name="psv", bufs=2)
                        if has_qkv_bias:
                            nc.tensor.matmul(
                                psv,
                                ones_tokb,
                                bv_sb[:, g * 512 : (g + 1) * 512],
                                start=True,
                                stop=False,
                            )
                        for di in range(ND):
                            nc.tensor.matmul(
                                psv,
                                xn1T[:, di, kt_i * 128 : (kt_i + 1) * 128],
                                wv_g[:, di, :],
                                start=(di == 0 and not has_qkv_bias),
                                stop=(di == ND - 1),
                            )
                        vt = wA.tile(
                            [128, 8, 65], BF16, tag=f"vt{kt_i}", name=f"vt{kt_i}"
                        )
                        nc.vector.memset(vt[:, :, 64:65], 1.0)
                        nc.vector.tensor_copy(
                            out=vt[:, :, 0:64],
                            in_=psv.rearrange("p (h d) -> p h d", h=8),
                        )
                        vt_tiles[kt_i] = vt

                wk_p = wA.tile([128, ND, 128], BF16, tag="wk_p", name="wk_p", bufs=2)
                nc.sync.dma_start(out=wk_p, in_=wk_t[:, :, p * 128 : (p + 1) * 128])
                wq_p = wA.tile([128, ND, 128], BF16, tag="wq_p", name="wq_p", bufs=2)
                nc.sync.dma_start(out=wq_p, in_=wq_t[:, :, p * 128 : (p + 1) * 128])

                ktp = wA.tile([128, T_CTX], BF16, tag="ktp", name="ktp", bufs=2)
                for ch in range(T_CTX // 512):
                    psk = psA.tile([128, 512], F32, tag="mm512", name="psk", bufs=2)
                    if has_qkv_bias:
                        nc.tensor.matmul(
                            psk,
                            bk_sb[:, p * 128 : (p + 1) * 128],
                            ones_512b,
                            start=True,
                            stop=False,
                        )
                    for di in range(ND):
                        nc.tensor.matmul(
                            psk,
                            wk_p[:, di, :],
                            xn1T[:, di, ch * 512 : (ch + 1) * 512],
                            start=(di == 0 and not has_qkv_bias),
                            stop=(di == ND - 1),
                        )
                    nc.vector.tensor_copy(out=ktp[:, ch * 512 : (ch + 1) * 512], in_=psk)

                qtp = wA.tile([128, T_OWN], BF16, tag="qtp", name="qtp", bufs=2)
                for ch in range(T_OWN // 512):
                    psq = psA.tile([128, 512], F32, tag="mm512", name="psq", bufs=2)
                    if has_qkv_bias:
                        nc.tensor.matmul(
                            psq,
                            bq_sb[:, p * 128 : (p + 1) * 128],
                            ones_512b,
                            start=True,
                            stop=False,
                        )
                    for di in range(ND):
                        nc.tensor.matmul(
                            psq,
                            wq_p[:, di, :],
                            xn1T[:, di, ch * 512 : (ch + 1) * 512],
                            start=(di == 0 and not has_qkv_bias),
                            stop=(di == ND - 1),
                        )
                    nc.vector.tensor_copy(out=qtp[:, ch * 512 : (ch + 1) * 512], in_=psq)

                for qc in range(T_OWN // 512 if "S" not in phases else 0):
                    q0 = qc * 512
                    ov = [
                        psA.tile([65, 512], F32, tag=f"ov{h}", name=f"ov{h}", bufs=1)
                        for h in range(2)
                    ]

                    def _s_exp(kt_i):
                        st = psA.tile(
                            [128, 2, 512], F32, tag="stpair", name="st", bufs=2
                        )
                        for h in range(2):
                            nc.tensor.matmul(
                                st[:, h, :],
                                ktp[h * 64 : (h + 1) * 64, kt_i * 128 : (kt_i + 1) * 128],
                                qtp[h * 64 : (h + 1) * 64, q0 : q0 + 512],
                                start=True,
                                stop=True,
                            )
                        ptm = wA.tile(
                            [128, 2, 512], BF16, tag="ptm", name="ptm", bufs=4
                        )
                        nc.scalar.activation(out=ptm, in_=st, func=AF.Exp, scale=0.125)
                        return [ptm[:, 0, :], ptm[:, 1, :]]

                    # software-pipeline: keep one S/exp in flight ahead of PV
                    # so the PE never stalls on the ACT exp of the current tile
                    pts_prev = _s_exp(0)
                    for kt_i in range(NT_CTX):
                        pts_next = _s_exp(kt_i + 1) if kt_i + 1 < NT_CTX else None
                        for h in range(2):
                            nc.tensor.matmul(
                                ov[h],
                                vt_tiles[kt_i][:, 2 * (p % 4) + h, :],
                                pts_prev[h],
                                start=(kt_i == 0),
                                stop=(kt_i == NT_CTX - 1),
                            )
                        pts_prev = pts_next
                    for h in range(2):
                        ovsb = wA.tile([65, 512], F32, tag="ovsb", name="ovsb", bufs=2)
                        nc.vector.tensor_copy(out=ovsb, in_=ov[h])
                        rec = wA.tile([1, 512], F32R, tag="rec", name="rec", bufs=2)
                        with nc.allow_low_precision(reason="f32r softmax denom"):
                            nc.vector.reciprocal(out=rec, in_=ovsb[64:65, :])
                        bc = psA.tile([64, 512], F32, tag="mm512", name="bc", bufs=2)
                        nc.tensor.matmul(bc, ones_h, rec, start=True, stop=True)
                        nc.vector.tensor_tensor(
                            out=attnT[h * 64 : (h + 1) * 64, p, q0 : q0 + 512],
                            in0=ovsb[0:64, :],
                            in1=bc,
                            op=OP.mult,
                        )


        # ------- Phase B+C: proj + residual -> x2; LN2 + transpose -> xn2T -------
        if "B" in phases:
         with (
            tc.tile_pool(name="wB", bufs=3) as wB,
            tc.tile_pool(name="psB", bufs=4, space="PSUM") as psB,
            tc.tile_pool(name="psC", bufs=3, space="PSUM") as psC,
        ):
            wp_sb = wB.tile([128, ND, D], BF16, tag="wp_sb", name="wp_sb", bufs=1)
            nc.sync.dma_start(out=wp_sb, in_=wp_t)
            for tt4 in range(NT_OWN // 4):
              xr4 = wB.tile([128, 4, D], F32, tag="xr_t", name="xr_t", bufs=2)
              nc.sync.dma_start(
                  out=xr4,
                  in_=x_ctx.ap()[tt4 * 512 : (tt4 + 1) * 512, :].rearrange(
                      "(f p) d -> p f d", p=128
                  ),
              )
              for ti in range(4):
                tt = tt4 * 4 + ti
                xr_t = xr4[:, ti, :]
                for ch in range(2):
                    psb = psB.tile([128, 512], F32, tag="psb", name="psb")
                    nc.tensor.matmul(
                        psb,
                        ones_tok,
                        bp_sb[:, ch * 512 : (ch + 1) * 512],
                        start=True,
                        stop=False,
                    )
                    for di in range(ND):
                        nc.tensor.matmul(
                            psb,
                            attnT[:, di, tt * 128 : (tt + 1) * 128],
                            wp_sb[:, di, ch * 512 : (ch + 1) * 512],
                            start=False,
                            stop=(di == ND - 1),
                        )
                    nc.vector.tensor_tensor(
                        out=x2[:, tt, ch * 512 : (ch + 1) * 512],
                        in0=psb,
                        in1=xr_t[:, ch * 512 : (ch + 1) * 512],
                        op=OP.add,
                    )
                if "C" in phases:
                    xn2 = wB.tile([128, D], BF16, tag="p2_xn", name="p2_xn")
                    _layernorm_tiles(nc, wB, eps_t, x2[:, tt, :], xn2)
                    for dt in range(ND):
                        tp2 = psC.tile([128, 128], BF16, tag="p2_tp", name="p2_tp")
                        nc.tensor.transpose(
                            tp2, xn2[:, dt * 128 : (dt + 1) * 128], ident_b
                        )
                        dst = xn2T[:, dt, tt * 128 : (tt + 1) * 128]
                        if dt % 2 == 0:
                            nc.vector.tensor_copy(out=dst, in_=tp2)
                        else:
                            nc.scalar.copy(out=dst, in_=tp2)

        p_big1_cm.__exit__(None, None, None)  # free xn1T + attnT

        # ---------------- Phase D: MLP + residual -> y ----------------
        if "D" in phases:
         with (
            tc.tile_pool(name="wD", bufs=3) as wD,
            tc.tile_pool(name="h1pool", bufs=1) as h1pool,
            tc.tile_pool(name="psD1", bufs=3, space="PSUM") as psD1,
            tc.tile_pool(name="psD2", bufs=1, space="PSUM") as psD2,
        ):
            for c2 in range(2):
                t0 = c2 * 512
                h1 = h1pool.tile([128, NH1, 512], BF16, tag="h1", name="h1", bufs=2)
                for ht4 in range(NH1 // 4):
                    w1s = wD.tile(
                        [128, ND, 4, 128], BF16, tag="w1s", name="w1s", bufs=2
                    )
                    nc.sync.dma_start(
                        out=w1s,
                        in_=w1_ap[:, :, ht4 * 512 : (ht4 + 1) * 512].rearrange(
                            "p t (f o) -> p t f o", f=4
                        ),
                    )
                    for hi in range(4):
                        ht = ht4 * 4 + hi
                        psh = psD1.tile([128, 512], F32, tag="psh", name="psh")
                        for di in range(ND):
                            nc.tensor.matmul(
                                psh,
                                w1s[:, di, hi, :],
                                xn2T[:, di, t0 : t0 + 512],
                                start=(di == 0),
                                stop=(di == ND - 1),
                            )
                        nc.scalar.activation(
                            out=h1[:, ht, :],
                            in_=psh,
                            func=AF.Gelu,
                            bias=b1t_sb[:, ht : ht + 1],
                            scale=1.0,
                        )
                for dch in range(2):
                    out_ps = [
                        psD2.tile([128, 512], F32, tag=f"o{i}", name=f"o{i}")
                        for i in range(4)
                    ]
                    for ts in range(4):
                        nc.tensor.matmul(
                            out_ps[ts],
                            ones_tok,
                            b2_sb[:, dch * 512 : (dch + 1) * 512],
                            start=True,
                            stop=False,
                        )
                    for ht4 in range(NH1 // 4):
                        w2s = wD.tile(
                            [128, 4, 512], BF16, tag="w2s", name="w2s", bufs=3
                        )
                        nc.sync.dma_start(
                            out=w2s,
                            in_=w2_ap[:, ht4 * 4 : (ht4 + 1) * 4, dch * 512 : (dch + 1) * 512],
                        )
                        for hi in range(4):
                            ht = ht4 * 4 + hi
                            for ts in range(4):
                                nc.tensor.matmul(
                                    out_ps[ts],
                                    h1[:, ht, ts * 128 : (ts + 1) * 128],
                                    w2s[:, hi, :],
                                    start=False,
                                    stop=(ht == NH1 - 1),
                                )
                    for ts in range(4):
                        tt = c2 * 4 + ts
                        yt = wD.tile([128, 512], F32, tag="yt", name="yt")
                        nc.vector.tensor_tensor(
                            out=yt,
                            in0=out_ps[ts],
                            in1=x2[:, tt, dch * 512 : (dch + 1) * 512],
                            op=OP.add,
                        )
                        nc.sync.dma_start(
                            out=y_d.ap()[
                                tt * 128 : (tt + 1) * 128,
                                dch * 512 : (dch + 1) * 512,
                            ],
                            in_=yt,
                        )

        p_xn2_cm.__exit__(None, None, None)
        gpersist_cm.__exit__(None, None, None)
        const_cm.__exit__(None, None, None)

    nc.compile()
    return nc


def _get_nc(has_qkv_bias: bool):
    key = ("v3", has_qkv_bias)
    if key not in _BUILD_CACHE:
        _BUILD_CACHE[key] = _build(has_qkv_bias)
    return _BUILD_CACHE[key]


# per-core inputs are sharded over the core mesh axis; everything else is
# broadcast once instead of being concatenated 8x (saves ~300MB of host->
# device transfer per call)
_SHARDED_INPUTS = {"x_ctx"}
_RUNNER_CACHE = {}


def _get_runner(has_qkv_bias: bool):
    key = has_qkv_bias
    if key in _RUNNER_CACHE:
        return _RUNNER_CACHE[key]

    import jax
    from jax.experimental.shard_map import shard_map
    from jax.sharding import Mesh, NamedSharding, PartitionSpec

    from concourse import bass2jax

    nc = _get_nc(has_qkv_bias)
    bass2jax.install_neuronx_cc_hook()
    partition_name = nc.partition_id_tensor.name if nc.partition_id_tensor else None

    in_names, out_names, out_avals, zero_outs = [], [], [], []
    for alloc in nc.m.functions[0].allocations:
        if not isinstance(alloc, mybir.MemoryLocationSet):
            continue
        name = alloc.memorylocations[0].name
        if alloc.kind == "ExternalInput":
            if name != partition_name:
                in_names.append(name)
        elif alloc.kind == "ExternalOutput":
            shape = tuple(alloc.tensor_shape)
            dtype = mybir.dt.np(alloc.dtype)
            out_names.append(name)
            out_avals.append(jax.core.ShapedArray(shape, dtype))
            zero_outs.append(np.zeros(shape, dtype))
    n_params = len(in_names)
    all_in_names = in_names + out_names
    if partition_name is not None:
        all_in_names.append(partition_name)

    def _body(*args):
        operands = list(args)
        if partition_name is not None:
            operands.append(bass2jax.partition_id_tensor())
        outs = bass2jax._bass_exec_p.bind(
            *operands,
            out_avals=tuple(out_avals),
            in_names=tuple(all_in_names),
            out_names=tuple(out_names),
            lowering_input_output_aliases=(),
            sim_require_finite=True,
            sim_require_nnan=True,
            nc=nc,
        )
        return tuple(outs)

    devices = jax.devices()[:NCORES]
    mesh = Mesh(np.asarray(devices), ("core",))
    core_spec = PartitionSpec("core")
    rep_spec = PartitionSpec()
    in_specs = tuple(
        core_spec if n in _SHARDED_INPUTS else rep_spec for n in in_names
    ) + (core_spec,) * len(out_names)
    out_specs = (core_spec,) * len(out_names)
    fn = jax.jit(
        shard_map(
            _body, mesh=mesh, in_specs=in_specs, out_specs=out_specs, check_rep=False
        ),
        keep_unused=True,
    )
    dev_zeros = [
        jax.device_put(
            np.zeros((NCORES * z.shape[0], *z.shape[1:]), z.dtype),
            NamedSharding(mesh, core_spec),
        )
        for z in zero_outs
    ]
    runner = {
        "fn": fn,
        "in_names": in_names,
        "out_names": out_names,
        "zero_outs": zero_outs,
        "dev_zeros": dev_zeros,
        "mesh": mesh,
        "core_spec": core_spec,
        "rep_spec": rep_spec,
        "NamedSharding": NamedSharding,
        "jax": jax,
    }
    _RUNNER_CACHE[key] = runner
    return runner


def kernel(x, w_qkv, w_proj, b_proj, w1, b1, w2, b2, g1, be1, g2, be2, **_):
    x = np.ascontiguousarray(np.asarray(x, dtype=np.float32))
    w_qkv = np.asarray(w_qkv, dtype=np.float32)
    w_proj = np.asarray(w_proj, dtype=np.float32)
    b_proj = np.asarray(b_proj, dtype=np.float32)
    w1 = np.asarray(w1, dtype=np.float32)
    b1 = np.asarray(b1, dtype=np.float32)
    w2 = np.asarray(w2, dtype=np.float32)
    b2 = np.asarray(b2, dtype=np.float32)
    g1 = np.asarray(g1, dtype=np.float32)
    be1 = np.asarray(be1, dtype=np.float32)
    g2 = np.asarray(g2, dtype=np.float32)
    be2 = np.asarray(be2, dtype=np.float32)

    # fold LN affines into following matmuls (exact)
    wqkv_eff = w_qkv * g1[:, None]
    qkv_bias = be1 @ w_qkv
    bf = ml_dtypes.bfloat16
    inputs = {
        "wq": np.ascontiguousarray(wqkv_eff[:, :D].astype(bf)),
        "wk": np.ascontiguousarray(wqkv_eff[:, D : 2 * D].astype(bf)),
        "wv": np.ascontiguousarray(wqkv_eff[:, 2 * D :].astype(bf)),
        "wproj": np.ascontiguousarray(w_proj.astype(bf)),
        "w1": np.ascontiguousarray((w1 * g2[:, None]).astype(bf)),
        "w2": np.ascontiguousarray(w2.astype(bf)),
        "b1t": np.ascontiguousarray((b1 + be2 @ w1).reshape(NH1, 128).T),
        "b2": np.ascontiguousarray(b2[None, :]),
        "bp": np.ascontiguousarray(b_proj[None, :]),
    }
    has_qkv_bias = bool(np.any(qkv_bias != 0.0))
    if has_qkv_bias:
        inputs["bq"] = np.ascontiguousarray(qkv_bias[None, :D].astype(bf))
        inputs["bk"] = np.ascontiguousarray(qkv_bias[None, D : 2 * D].astype(bf))
        inputs["bv"] = np.ascontiguousarray(qkv_bias[None, 2 * D :].astype(bf))

    # per-core context: own 1024 tokens first, then the rest of its batch's
    # sequence (key order permutation -- exact for softmax attention)
    xf = x.reshape(NCORES, T_OWN, D)
    parts = []
    for c in range(NCORES):
        other = xf[c ^ 1]
        parts.append(np.concatenate([xf[c], other], axis=0))
    inputs["x_ctx"] = np.ascontiguousarray(np.stack(parts).reshape(NCORES * T_CTX, D))

    global _last_host_inputs
    _last_host_inputs = inputs

    r = _get_runner(has_qkv_bias)
    jax = r["jax"]
    NamedSharding = r["NamedSharding"]
    dev_in = []
    for nname in r["in_names"]:
        spec = r["core_spec"] if nname in _SHARDED_INPUTS else r["rep_spec"]
        dev_in.append(
            jax.device_put(inputs[nname], NamedSharding(r["mesh"], spec))
        )
    dev_in.extend(r["dev_zeros"])
    outs = r["fn"](*dev_in)
    y = np.asarray(outs[r["out_names"].index("y")])
    global _LAST_RESULTS
    _LAST_RESULTS = outs
    return y.reshape(B, N, D)
